# revision 12
# baseline (speedup 1.0000x reference)
"""Trainium2 Bass kernel for nn_BoundaryPredictorMLP (segment-mean pooling MLP).

Sharding: pure data parallel — one batch sample per NeuronCore (B=8, 8 cores).

Pipeline (two device launches with a host step between — K is data-dependent):
  Launch 1 (device, fp32): G^T = (X @ W1)^T per core.  fp32 PE matmul keeps
    logit error ~1e-7 (boundary-threshold margin on this data is ~5e-5, so
    bf16-class matmuls would flip segment boundaries).
  Host: gelu (exact, erf-based) + logits + sigmoid via jax-CPU; boundary /
    segment-id bookkeeping in numpy mirroring the reference fp32 arithmetic.
  Launch 2 (device, fp32): segment-sum pooling as a one-hot matmul.  The
    one-hot matrix [S, K_pad] is built on device from masked seg-ids via
    iota + per-partition is_equal.  Only (s-chunk, k-chunk) pairs that can be
    non-zero on some core are emitted (segments are contiguous -> banded).
  Host: divide by counts, assemble the 5 reference outputs.
"""

import numpy as np

import concourse.bacc as bacc
import concourse.tile as tile
from concourse import mybir

F32 = mybir.dt.float32

B, S, D = 8, 2048, 768
NCORES = 8
SC = S // 128  # 16 s-chunks
DC = D // 128  # 6 d-chunks

_nc_cache = {}


def _run_spmd_cached(cache_key, nc, in_maps):
    """run_bass_via_pjrt with the jitted executable cached across calls.

    Mirrors concourse.bass2jax.run_bass_via_pjrt's multi-core path (shard_map
    over 8 devices, inputs concatenated on axis 0, donated zero output
    buffers) but keeps the compiled callable so repeat kernel() invocations
    skip retracing/recompiling.
    """
    import jax
    from jax.sharding import Mesh, PartitionSpec
    from jax.experimental.shard_map import shard_map
    from concourse import bass2jax, mybir as _mb

    n_cores = len(in_maps)
    entry = _nc_cache.get(("jit", cache_key))
    if entry is None:
        bass2jax.install_neuronx_cc_hook()
        in_names, out_names, out_avals = [], [], []
        partition_name = (
            nc.partition_id_tensor.name if nc.partition_id_tensor else None
        )
        for alloc in nc.m.functions[0].allocations:
            if not isinstance(alloc, _mb.MemoryLocationSet):
                continue
            name = alloc.memorylocations[0].name
            if alloc.kind == "ExternalInput":
                if name != partition_name:
                    in_names.append(name)
            elif alloc.kind == "ExternalOutput":
                out_names.append(name)
                out_avals.append(
                    jax.core.ShapedArray(
                        tuple(alloc.tensor_shape), _mb.dt.np(alloc.dtype)
                    )
                )
        n_params = len(in_names)
        n_outs = len(out_avals)
        all_in_names = list(in_names) + list(out_names)
        if partition_name is not None:
            all_in_names.append(partition_name)

        def _body(*args):
            operands = list(args)
            if partition_name is not None:
                operands.append(bass2jax.partition_id_tensor())
            outs = bass2jax._bass_exec_p.bind(
                *operands,
                out_avals=tuple(out_avals),
                in_names=tuple(all_in_names),
                out_names=tuple(out_names),
                lowering_input_output_aliases=(),
                sim_require_finite=True,
                sim_require_nnan=True,
                nc=nc,
            )
            return tuple(outs)

        devices = jax.devices()[:n_cores]
        mesh = Mesh(np.asarray(devices), ("core",))
        in_specs = (PartitionSpec("core"),) * (n_params + n_outs)
        out_specs = (PartitionSpec("core"),) * n_outs
        donate = tuple(range(n_params, n_params + n_outs))
        fn = jax.jit(
            shard_map(
                _body, mesh=mesh, in_specs=in_specs, out_specs=out_specs,
                check_rep=False,
            ),
            donate_argnums=donate,
            keep_unused=True,
        )
        entry = (fn, in_names, out_names, out_avals)
        _nc_cache[("jit", cache_key)] = entry

    fn, in_names, out_names, out_avals = entry
    concat_in = [
        np.concatenate([np.asarray(m[name]) for m in in_maps], axis=0)
        for name in in_names
    ]
    concat_zeros = [
        np.zeros((n_cores * a.shape[0], *a.shape[1:]), a.dtype) for a in out_avals
    ]
    out_arrs = fn(*concat_in, *concat_zeros)
    return [
        {
            name: np.asarray(out_arrs[i]).reshape(n_cores, *out_avals[i].shape)[c]
            for i, name in enumerate(out_names)
        }
        for c in range(n_cores)
    ]


def build_phase1():
    """Device MLP: logits[1,S] = gelu((X@W1)^T + b1) dot W2.

    The big GEMM runs in float32r (full-rate on PE); gelu+bias on ScalarE
    straight out of PSUM; the W2 contraction is a skinny fp32 matmul.  Rows
    with |logit| near the 0 threshold are exactly recomputed on the host, so
    f32r/LUT error cannot flip a boundary decision.
    """
    if "p1" in _nc_cache:
        return _nc_cache["p1"]
    F32R = mybir.dt.float32r
    nc = bacc.Bacc()
    xt = nc.declare_dram_parameter("xt", [D, S], F32, isOutput=False)
    w1 = nc.declare_dram_parameter("w1", [D, D], F32, isOutput=False)
    b1p = nc.declare_dram_parameter("b1", [D, 1], F32, isOutput=False)
    w2 = nc.declare_dram_parameter("w2", [D, 1], F32, isOutput=False)
    lg = nc.declare_dram_parameter("lg", [1, S], F32, isOutput=True)

    with tile.TileContext(nc) as tc:
        with (
            tc.tile_pool(name="sb", bufs=1) as sb,
            tc.tile_pool(name="ps", bufs=4, space="PSUM") as ps,
            tc.tile_pool(name="psl", bufs=2, space="PSUM") as psl,
        ):
            w1t = sb.tile([128, DC, D], F32R, tag="w1t")
            w1_r = w1[:, :].rearrange("(kc p) m -> p kc m", p=128)
            for kc in range(DC):
                nc.gpsimd.dma_start(out=w1t[:, kc, :], in_=w1_r[:, kc, :])
            w2t = sb.tile([128, DC], F32R, tag="w2t")
            nc.gpsimd.dma_start(
                out=w2t, in_=w2[:, :].rearrange("(kc p) one -> p (kc one)", p=128)
            )
            b1t = sb.tile([128, DC], F32, tag="b1t")
            nc.sync.dma_start(
                out=b1t, in_=b1p[:, :].rearrange("(mc p) one -> p (mc one)", p=128)
            )
            NS = S // 512
            xtt = sb.tile([128, DC, NS, 512], F32R, tag="xtt")
            xt_r = xt[:, :].rearrange("(kc p) (n s) -> p kc n s", p=128, s=512)
            for n in range(NS):
                # per-s-block DMA (with fp32->f32r cast) so the first matmul
                # group only waits for W1 + one slice, not the full transfer
                nc.gpsimd.dma_start(out=xtt[:, :, n, :], in_=xt_r[:, :, n, :])
            ht = sb.tile([128, DC, S], F32R, tag="ht")
            lt = sb.tile([1, S], F32, tag="lt")
            for n in range(NS):
                for mc in range(DC):
                    pt = ps.tile([128, 512], F32, tag="pt")
                    for kc in range(DC):
                        nc.tensor.matmul(
                            pt,
                            w1t[:, kc, mc * 128 : (mc + 1) * 128],
                            xtt[:, kc, n, :],
                            start=(kc == 0),
                            stop=(kc == DC - 1),
                        )
                    nc.scalar.activation(
                        out=ht[:, mc, n * 512 : (n + 1) * 512],
                        in_=pt,
                        func=mybir.ActivationFunctionType.Gelu,
                        bias=b1t[:, mc : mc + 1],
                        scale=1.0,
                    )
                pl = psl.tile([1, 512], F32, tag="pl")
                for mc in range(DC):
                    nc.tensor.matmul(
                        pl,
                        w2t[:, mc : mc + 1],
                        ht[:, mc, n * 512 : (n + 1) * 512],
                        start=(mc == 0),
                        stop=(mc == DC - 1),
                    )
                nc.vector.tensor_copy(lt[:, n * 512 : (n + 1) * 512], pl)
            nc.sync.dma_start(out=lg[:, :], in_=lt)
    nc.finalize()
    _nc_cache["p1"] = nc
    return nc


def build_phase2(k_pad, kc_sis):
    """Pooled_sum [K_pad, D] = onehot[S,K_pad]^T @ X[S,D], banded over kc_sis.

    kc_sis: tuple of (kc, (si, si, ...)) — which s-chunks feed each k-chunk.
    """
    key = ("p2", k_pad, kc_sis)
    if key in _nc_cache:
        return _nc_cache[key]
    nc = bacc.Bacc()
    x = nc.declare_dram_parameter("x", [S, D], F32, isOutput=False)
    msid = nc.declare_dram_parameter("msid", [128, SC], F32, isOutput=False)
    psum_out = nc.declare_dram_parameter("ps", [k_pad, D], F32, isOutput=True)

    # per-s-chunk k-band: contiguous kc cover of every pair this si is in
    si_band = {}
    for kc, sis in kc_sis:
        for si in sis:
            lo, hi = si_band.get(si, (kc, kc))
            si_band[si] = (min(lo, kc), max(hi, kc))
    bw = max(hi - lo + 1 for lo, hi in si_band.values())  # chunks

    with tile.TileContext(nc) as tc:
        with (
            tc.tile_pool(name="sb", bufs=1) as sb,
            tc.tile_pool(name="stg", bufs=4) as stg,
            tc.tile_pool(name="ps", bufs=4, space="PSUM") as ps,
        ):
            BF16 = mybir.dt.bfloat16
            xf = sb.tile([128, SC, D], F32, tag="xf")
            x_r = x[:, :].rearrange("(g si p) d -> p g si d", p=128, si=4)
            for g in range(SC // 4):
                nc.sync.dma_start(out=xf[:, 4 * g : 4 * g + 4, :], in_=x_r[:, g])
            xh = sb.tile([128, SC, D], BF16, tag="xh")
            xl = sb.tile([128, SC, D], BF16, tag="xl")
            for g in range(SC // 4):
                sl = slice(4 * g, 4 * g + 4)
                nc.scalar.copy(out=xh[:, sl, :], in_=xf[:, sl, :])
                nc.vector.tensor_tensor(
                    out=xl[:, sl, :], in0=xf[:, sl, :], in1=xh[:, sl, :],
                    op=mybir.AluOpType.subtract,
                )
            ms = sb.tile([128, SC], F32, tag="ms")
            nc.sync.dma_start(out=ms, in_=msid[:, :])
            it = sb.tile([128, k_pad], F32, tag="it")
            nc.gpsimd.iota(
                it,
                pattern=[[1, k_pad]],
                base=0,
                channel_multiplier=0,
                allow_small_or_imprecise_dtypes=True,
            )
            oh = sb.tile([128, SC, bw * 128], BF16, tag="oh")
            for si, (lo, hi) in sorted(si_band.items()):
                w = (hi - lo + 1) * 128
                nc.vector.tensor_scalar(
                    out=oh[:, si, 0:w],
                    in0=it[:, lo * 128 : lo * 128 + w],
                    scalar1=ms[:, si : si + 1],
                    scalar2=None,
                    op0=mybir.AluOpType.is_equal,
                )
            for kc, sis in kc_sis:
                pa = ps.tile([128, 512], F32, tag="pa")
                pb = ps.tile([128, 256], F32, tag="pb")
                n_si = len(sis)
                for j, si in enumerate(sis):
                    st, sp = (j == 0), (j == n_si - 1)
                    off = (kc - si_band[si][0]) * 128
                    lhs = oh[:, si, off : off + 128]
                    nc.tensor.matmul(pa, lhs, xh[:, si, 0:512], start=st, stop=False)
                    nc.tensor.matmul(pa, lhs, xl[:, si, 0:512], start=False, stop=sp)
                    nc.tensor.matmul(pb, lhs, xh[:, si, 512:768], start=st, stop=False)
                    nc.tensor.matmul(pb, lhs, xl[:, si, 512:768], start=False, stop=sp)
                oa = stg.tile([128, D], F32, tag="oa")
                nc.vector.tensor_copy(oa[:, 0:512], pa)
                nc.vector.tensor_copy(oa[:, 512:768], pb)
                nc.sync.dma_start(
                    out=psum_out[kc * 128 : (kc + 1) * 128, :], in_=oa
                )
    nc.finalize()
    _nc_cache[key] = nc
    return nc


# |logit| band inside which the host exactly recomputes the MLP row.  Device
# logit error (f32r GEMM + gelu LUT) is measured at ~1e-4 on this workload;
# 1e-2 gives a ~100x safety margin at ~2% of rows recomputed.
TAU = np.float32(1e-2)


def _exact_rows(hidden, W1, b1, W2, b2, bb, ss):
    """Exact fp32 reference MLP for selected (batch, seq) rows."""
    import math

    from scipy.special import erf

    Xrows = hidden[bb, ss, :].astype(np.float32)  # [R, D]
    Grows = (Xrows @ W1 + b1).astype(np.float64)
    hrows = (Grows * 0.5 * (1.0 + erf(Grows / math.sqrt(2.0)))).astype(np.float32)
    return (hrows @ W2 + b2).astype(np.float32)


def kernel(hidden, lengths, W1, b1, W2, b2):
    hidden = np.ascontiguousarray(hidden, dtype=np.float32)
    lengths = np.asarray(lengths, dtype=np.float32)
    W1 = np.ascontiguousarray(W1, dtype=np.float32)
    b1 = np.asarray(b1, dtype=np.float32)
    W2 = np.asarray(W2, dtype=np.float32)
    b2 = np.asarray(b2, dtype=np.float32)

    # ---------------- Phase 1: logits on device ----------------
    nc1 = build_phase1()
    w2c = np.ascontiguousarray(W2.reshape(D, 1))
    b1c = np.ascontiguousarray(b1.reshape(D, 1))
    in_maps1 = [
        {"xt": np.ascontiguousarray(hidden[b].T), "w1": W1, "b1": b1c, "w2": w2c}
        for b in range(B)
    ]
    res1 = _run_spmd_cached("p1", nc1, in_maps1)
    logits = np.stack([res1[b]["lg"].reshape(S) for b in range(B)])  # [B,S]
    logits = (logits + b2).astype(np.float32)

    # exact host recompute of rows near the decision threshold
    band = np.abs(logits) < TAU
    if band.any():
        bb, ss = np.nonzero(band)
        logits[bb, ss] = _exact_rows(hidden, W1, b1, W2, b2, bb, ss)

    with np.errstate(over="ignore"):
        probs = (np.float32(1.0) / (np.float32(1.0) + np.exp(-logits))).astype(
            np.float32
        )

    actual_lens = (lengths * np.float32(S)).astype(np.int32)  # [B]
    sidx = np.arange(S, dtype=np.int64)
    valid = (sidx[None, :] < actual_lens[:, None]).astype(np.float32)  # [B,S]
    soft = probs * valid
    hard = (probs > np.float32(0.5)).astype(np.float32) * valid
    last_valid = np.clip(actual_lens - 1, 0, S - 1)
    bi = np.arange(B)
    soft[bi, last_valid] = np.float32(1.0)
    hard[bi, last_valid] = np.float32(1.0)
    hard_b = (hard - soft) + soft  # exact == hard in fp32; mirrors reference
    K = int(np.max(np.sum(hard_b, axis=1)))
    seg_id = np.cumsum(hard_b, axis=1, dtype=np.float32) - hard_b  # [B,S]
    nb = np.sum(hard_b * valid, axis=1)  # [B] f32

    masked_probs = probs * valid

    max_segments = max(K, 1)
    full = nb >= max_segments - 1
    partial = (nb > 0) & (nb < max_segments - 1)
    shortened = np.where(
        full,
        np.float32(1.0),
        np.where(partial, (nb + np.float32(1.0)) / np.float32(max_segments),
                 np.float32(0.0)),
    ).astype(np.float32)

    # ---------------- Phase 2: banded one-hot segment-sum ----------------
    k_pad = ((K + 127) // 128) * 128
    msid = np.where(valid > 0, seg_id, np.float32(-1.0)).astype(np.float32)

    kc_map = {}
    for b in range(B):
        al = int(actual_lens[b])
        for si in range(SC):
            s0 = si * 128
            if s0 >= al:
                break
            s1 = min(s0 + 128, al)
            lo = int(seg_id[b, s0])
            hi = int(seg_id[b, s1 - 1])
            for kc in range(lo // 128, hi // 128 + 1):
                kc_map.setdefault(kc, set()).add(si)
    kc_sis = tuple(
        (kc, tuple(sorted(kc_map[kc]))) for kc in sorted(kc_map)
    )

    nc2 = build_phase2(k_pad, kc_sis)
    in_maps2 = [
        {"x": hidden[b],
         "msid": np.ascontiguousarray(msid[b].reshape(SC, 128).T)}
        for b in range(B)
    ]
    global _last_in_maps1, _last_in_maps2
    _last_in_maps1 = in_maps1
    _last_in_maps2 = in_maps2
    res2 = _run_spmd_cached(("p2", k_pad, kc_sis), nc2, in_maps2)
    pooled_sum = np.stack([res2[b]["ps"][:K] for b in range(B)])  # [B,K,D]

    counts = np.zeros((B, K), dtype=np.float32)
    for b in range(B):
        v = valid[b] > 0
        ids = seg_id[b, v].astype(np.int64)
        if ids.size:
            cnt = np.bincount(ids, minlength=K)
            counts[b] = cnt[:K]
    counts = np.maximum(counts, np.float32(1.0))
    pooled = pooled_sum / counts[:, :, None]

    return (
        pooled.astype(np.float32),
        masked_probs.astype(np.float32),
        shortened,
        nb.astype(np.float32),
        actual_lens.astype(np.float32),
    )


# revision 13
# speedup vs baseline: 1.0140x; 1.0140x over previous
"""Trainium2 Bass kernel for nn_BoundaryPredictorMLP (segment-mean pooling MLP).

Sharding: pure data parallel — one batch sample per NeuronCore (B=8, 8 cores).

Pipeline (two device launches with a host step between — K is data-dependent):
  Launch 1 (device, fp32): G^T = (X @ W1)^T per core.  fp32 PE matmul keeps
    logit error ~1e-7 (boundary-threshold margin on this data is ~5e-5, so
    bf16-class matmuls would flip segment boundaries).
  Host: gelu (exact, erf-based) + logits + sigmoid via jax-CPU; boundary /
    segment-id bookkeeping in numpy mirroring the reference fp32 arithmetic.
  Launch 2 (device, fp32): segment-sum pooling as a one-hot matmul.  The
    one-hot matrix [S, K_pad] is built on device from masked seg-ids via
    iota + per-partition is_equal.  Only (s-chunk, k-chunk) pairs that can be
    non-zero on some core are emitted (segments are contiguous -> banded).
  Host: divide by counts, assemble the 5 reference outputs.
"""

import numpy as np

import concourse.bacc as bacc
import concourse.tile as tile
from concourse import mybir

F32 = mybir.dt.float32

B, S, D = 8, 2048, 768
NCORES = 8
SC = S // 128  # 16 s-chunks
DC = D // 128  # 6 d-chunks

_nc_cache = {}


def _run_spmd_cached(cache_key, nc, in_maps):
    """run_bass_via_pjrt with the jitted executable cached across calls.

    Mirrors concourse.bass2jax.run_bass_via_pjrt's multi-core path (shard_map
    over 8 devices, inputs concatenated on axis 0, donated zero output
    buffers) but keeps the compiled callable so repeat kernel() invocations
    skip retracing/recompiling.
    """
    import jax
    from jax.sharding import Mesh, PartitionSpec
    from jax.experimental.shard_map import shard_map
    from concourse import bass2jax, mybir as _mb

    n_cores = len(in_maps)
    entry = _nc_cache.get(("jit", cache_key))
    if entry is None:
        bass2jax.install_neuronx_cc_hook()
        in_names, out_names, out_avals = [], [], []
        partition_name = (
            nc.partition_id_tensor.name if nc.partition_id_tensor else None
        )
        for alloc in nc.m.functions[0].allocations:
            if not isinstance(alloc, _mb.MemoryLocationSet):
                continue
            name = alloc.memorylocations[0].name
            if alloc.kind == "ExternalInput":
                if name != partition_name:
                    in_names.append(name)
            elif alloc.kind == "ExternalOutput":
                out_names.append(name)
                out_avals.append(
                    jax.core.ShapedArray(
                        tuple(alloc.tensor_shape), _mb.dt.np(alloc.dtype)
                    )
                )
        n_params = len(in_names)
        n_outs = len(out_avals)
        all_in_names = list(in_names) + list(out_names)
        if partition_name is not None:
            all_in_names.append(partition_name)

        def _body(*args):
            operands = list(args)
            if partition_name is not None:
                operands.append(bass2jax.partition_id_tensor())
            outs = bass2jax._bass_exec_p.bind(
                *operands,
                out_avals=tuple(out_avals),
                in_names=tuple(all_in_names),
                out_names=tuple(out_names),
                lowering_input_output_aliases=(),
                sim_require_finite=True,
                sim_require_nnan=True,
                nc=nc,
            )
            return tuple(outs)

        devices = jax.devices()[:n_cores]
        mesh = Mesh(np.asarray(devices), ("core",))
        in_specs = (PartitionSpec("core"),) * (n_params + n_outs)
        out_specs = (PartitionSpec("core"),) * n_outs
        donate = tuple(range(n_params, n_params + n_outs))
        fn = jax.jit(
            shard_map(
                _body, mesh=mesh, in_specs=in_specs, out_specs=out_specs,
                check_rep=False,
            ),
            donate_argnums=donate,
            keep_unused=True,
        )
        entry = (fn, in_names, out_names, out_avals)
        _nc_cache[("jit", cache_key)] = entry

    fn, in_names, out_names, out_avals = entry
    concat_in = [
        np.concatenate([np.asarray(m[name]) for m in in_maps], axis=0)
        for name in in_names
    ]
    concat_zeros = [
        np.zeros((n_cores * a.shape[0], *a.shape[1:]), a.dtype) for a in out_avals
    ]
    out_arrs = fn(*concat_in, *concat_zeros)
    return [
        {
            name: np.asarray(out_arrs[i]).reshape(n_cores, *out_avals[i].shape)[c]
            for i, name in enumerate(out_names)
        }
        for c in range(n_cores)
    ]


def build_phase1():
    """Device MLP: logits[1,S] = gelu((X@W1)^T + b1) dot W2.

    The big GEMM runs in float32r (full-rate on PE); gelu+bias on ScalarE
    straight out of PSUM; the W2 contraction is a skinny fp32 matmul.  Rows
    with |logit| near the 0 threshold are exactly recomputed on the host, so
    f32r/LUT error cannot flip a boundary decision.
    """
    if "p1" in _nc_cache:
        return _nc_cache["p1"]
    F32R = mybir.dt.float32r
    nc = bacc.Bacc()
    xt = nc.declare_dram_parameter("xt", [D, S], F32, isOutput=False)
    w1 = nc.declare_dram_parameter("w1", [D, D], F32, isOutput=False)
    b1p = nc.declare_dram_parameter("b1", [D, 1], F32, isOutput=False)
    w2 = nc.declare_dram_parameter("w2", [D, 1], F32, isOutput=False)
    lg = nc.declare_dram_parameter("lg", [1, S], F32, isOutput=True)

    with tile.TileContext(nc) as tc:
        with (
            tc.tile_pool(name="sb", bufs=1) as sb,
            tc.tile_pool(name="ps", bufs=4, space="PSUM") as ps,
            tc.tile_pool(name="psl", bufs=2, space="PSUM") as psl,
        ):
            w1t = sb.tile([128, DC, D], F32R, tag="w1t")
            w1_r = w1[:, :].rearrange("(kc p) m -> p kc m", p=128)
            for kc in range(DC):
                nc.gpsimd.dma_start(out=w1t[:, kc, :], in_=w1_r[:, kc, :])
            w2t = sb.tile([128, DC], F32R, tag="w2t")
            nc.gpsimd.dma_start(
                out=w2t, in_=w2[:, :].rearrange("(kc p) one -> p (kc one)", p=128)
            )
            b1t = sb.tile([128, DC], F32, tag="b1t")
            nc.sync.dma_start(
                out=b1t, in_=b1p[:, :].rearrange("(mc p) one -> p (mc one)", p=128)
            )
            NS = S // 512
            xtt = sb.tile([128, DC, NS, 512], F32R, tag="xtt")
            xt_r = xt[:, :].rearrange("(kc p) (n s) -> p kc n s", p=128, s=512)
            for n in range(NS):
                # per-s-block DMA (with fp32->f32r cast) so the first matmul
                # group only waits for W1 + one slice, not the full transfer
                nc.gpsimd.dma_start(out=xtt[:, :, n, :], in_=xt_r[:, :, n, :])
            ht = sb.tile([128, DC, S], F32R, tag="ht")
            lt = sb.tile([1, S], F32, tag="lt")
            def emit_mlp2(n):
                pl = psl.tile([1, 512], F32, tag="pl")
                for mc in range(DC):
                    nc.tensor.matmul(
                        pl,
                        w2t[:, mc : mc + 1],
                        ht[:, mc, n * 512 : (n + 1) * 512],
                        start=(mc == 0),
                        stop=(mc == DC - 1),
                    )
                nc.vector.tensor_copy(lt[:, n * 512 : (n + 1) * 512], pl)

            for n in range(NS):
                for mc in range(DC):
                    pt = ps.tile([128, 512], F32, tag="pt")
                    for kc in range(DC):
                        nc.tensor.matmul(
                            pt,
                            w1t[:, kc, mc * 128 : (mc + 1) * 128],
                            xtt[:, kc, n, :],
                            start=(kc == 0),
                            stop=(kc == DC - 1),
                        )
                    nc.scalar.activation(
                        out=ht[:, mc, n * 512 : (n + 1) * 512],
                        in_=pt,
                        func=mybir.ActivationFunctionType.Gelu,
                        bias=b1t[:, mc : mc + 1],
                        scale=1.0,
                    )
                if n >= 1:
                    emit_mlp2(n - 1)
            emit_mlp2(NS - 1)
            nc.sync.dma_start(out=lg[:, :], in_=lt)
    nc.finalize()
    _nc_cache["p1"] = nc
    return nc


def build_phase2(k_pad, kc_sis):
    """Pooled_sum [K_pad, D] = onehot[S,K_pad]^T @ X[S,D], banded over kc_sis.

    kc_sis: tuple of (kc, (si, si, ...)) — which s-chunks feed each k-chunk.
    """
    key = ("p2", k_pad, kc_sis)
    if key in _nc_cache:
        return _nc_cache[key]
    BF16 = mybir.dt.bfloat16
    nc = bacc.Bacc()
    xhp = nc.declare_dram_parameter("xh", [S, D], BF16, isOutput=False)
    xlp = nc.declare_dram_parameter("xl", [S, D], BF16, isOutput=False)
    msid = nc.declare_dram_parameter("msid", [128, SC], F32, isOutput=False)
    psum_out = nc.declare_dram_parameter("ps", [k_pad, D], F32, isOutput=True)

    # per-s-chunk k-band: contiguous kc cover of every pair this si is in
    si_band = {}
    for kc, sis in kc_sis:
        for si in sis:
            lo, hi = si_band.get(si, (kc, kc))
            si_band[si] = (min(lo, kc), max(hi, kc))
    bw = max(hi - lo + 1 for lo, hi in si_band.values())  # chunks

    with tile.TileContext(nc) as tc:
        with (
            tc.tile_pool(name="sb", bufs=1) as sb,
            tc.tile_pool(name="stg", bufs=4) as stg,
            tc.tile_pool(name="ps", bufs=4, space="PSUM") as ps,
        ):
            xh = sb.tile([128, SC, D], BF16, tag="xh")
            xl = sb.tile([128, SC, D], BF16, tag="xl")
            xh_r = xhp[:, :].rearrange("(g si p) d -> p g si d", p=128, si=4)
            xl_r = xlp[:, :].rearrange("(g si p) d -> p g si d", p=128, si=4)
            for g in range(SC // 4):
                nc.sync.dma_start(out=xh[:, 4 * g : 4 * g + 4, :], in_=xh_r[:, g])
                nc.sync.dma_start(out=xl[:, 4 * g : 4 * g + 4, :], in_=xl_r[:, g])
            ms = sb.tile([128, SC], F32, tag="ms")
            nc.sync.dma_start(out=ms, in_=msid[:, :])
            it = sb.tile([128, k_pad], F32, tag="it")
            nc.gpsimd.iota(
                it,
                pattern=[[1, k_pad]],
                base=0,
                channel_multiplier=0,
                allow_small_or_imprecise_dtypes=True,
            )
            oh = sb.tile([128, SC, bw * 128], BF16, tag="oh")
            for si, (lo, hi) in sorted(si_band.items()):
                w = (hi - lo + 1) * 128
                nc.vector.tensor_scalar(
                    out=oh[:, si, 0:w],
                    in0=it[:, lo * 128 : lo * 128 + w],
                    scalar1=ms[:, si : si + 1],
                    scalar2=None,
                    op0=mybir.AluOpType.is_equal,
                )
            for kc, sis in kc_sis:
                pa = ps.tile([128, 512], F32, tag="pa")
                pb = ps.tile([128, 256], F32, tag="pb")
                n_si = len(sis)
                for j, si in enumerate(sis):
                    st, sp = (j == 0), (j == n_si - 1)
                    off = (kc - si_band[si][0]) * 128
                    lhs = oh[:, si, off : off + 128]
                    nc.tensor.matmul(pa, lhs, xh[:, si, 0:512], start=st, stop=False)
                    nc.tensor.matmul(pa, lhs, xl[:, si, 0:512], start=False, stop=sp)
                    nc.tensor.matmul(pb, lhs, xh[:, si, 512:768], start=st, stop=False)
                    nc.tensor.matmul(pb, lhs, xl[:, si, 512:768], start=False, stop=sp)
                oa = stg.tile([128, D], F32, tag="oa")
                nc.vector.tensor_copy(oa[:, 0:512], pa)
                nc.vector.tensor_copy(oa[:, 512:768], pb)
                nc.sync.dma_start(
                    out=psum_out[kc * 128 : (kc + 1) * 128, :], in_=oa
                )
    nc.finalize()
    _nc_cache[key] = nc
    return nc


# |logit| band inside which the host exactly recomputes the MLP row.  Device
# logit error (f32r GEMM + gelu LUT) is measured at ~1e-4 on this workload;
# 1e-2 gives a ~100x safety margin at ~2% of rows recomputed.
TAU = np.float32(1e-2)


def _exact_rows(hidden, W1, b1, W2, b2, bb, ss):
    """Exact fp32 reference MLP for selected (batch, seq) rows."""
    import math

    from scipy.special import erf

    Xrows = hidden[bb, ss, :].astype(np.float32)  # [R, D]
    Grows = (Xrows @ W1 + b1).astype(np.float64)
    hrows = (Grows * 0.5 * (1.0 + erf(Grows / math.sqrt(2.0)))).astype(np.float32)
    return (hrows @ W2 + b2).astype(np.float32)


def kernel(hidden, lengths, W1, b1, W2, b2):
    hidden = np.ascontiguousarray(hidden, dtype=np.float32)
    lengths = np.asarray(lengths, dtype=np.float32)
    W1 = np.ascontiguousarray(W1, dtype=np.float32)
    b1 = np.asarray(b1, dtype=np.float32)
    W2 = np.asarray(W2, dtype=np.float32)
    b2 = np.asarray(b2, dtype=np.float32)

    # ---------------- Phase 1: logits on device ----------------
    nc1 = build_phase1()
    w2c = np.ascontiguousarray(W2.reshape(D, 1))
    b1c = np.ascontiguousarray(b1.reshape(D, 1))
    in_maps1 = [
        {"xt": np.ascontiguousarray(hidden[b].T), "w1": W1, "b1": b1c, "w2": w2c}
        for b in range(B)
    ]
    res1 = _run_spmd_cached("p1", nc1, in_maps1)
    logits = np.stack([res1[b]["lg"].reshape(S) for b in range(B)])  # [B,S]
    logits = (logits + b2).astype(np.float32)

    # exact host recompute of rows near the decision threshold
    band = np.abs(logits) < TAU
    if band.any():
        bb, ss = np.nonzero(band)
        logits[bb, ss] = _exact_rows(hidden, W1, b1, W2, b2, bb, ss)

    with np.errstate(over="ignore"):
        probs = (np.float32(1.0) / (np.float32(1.0) + np.exp(-logits))).astype(
            np.float32
        )

    actual_lens = (lengths * np.float32(S)).astype(np.int32)  # [B]
    sidx = np.arange(S, dtype=np.int64)
    valid = (sidx[None, :] < actual_lens[:, None]).astype(np.float32)  # [B,S]
    soft = probs * valid
    hard = (probs > np.float32(0.5)).astype(np.float32) * valid
    last_valid = np.clip(actual_lens - 1, 0, S - 1)
    bi = np.arange(B)
    soft[bi, last_valid] = np.float32(1.0)
    hard[bi, last_valid] = np.float32(1.0)
    hard_b = (hard - soft) + soft  # exact == hard in fp32; mirrors reference
    K = int(np.max(np.sum(hard_b, axis=1)))
    seg_id = np.cumsum(hard_b, axis=1, dtype=np.float32) - hard_b  # [B,S]
    nb = np.sum(hard_b * valid, axis=1)  # [B] f32

    masked_probs = probs * valid

    max_segments = max(K, 1)
    full = nb >= max_segments - 1
    partial = (nb > 0) & (nb < max_segments - 1)
    shortened = np.where(
        full,
        np.float32(1.0),
        np.where(partial, (nb + np.float32(1.0)) / np.float32(max_segments),
                 np.float32(0.0)),
    ).astype(np.float32)

    # ---------------- Phase 2: banded one-hot segment-sum ----------------
    k_pad = ((K + 127) // 128) * 128
    msid = np.where(valid > 0, seg_id, np.float32(-1.0)).astype(np.float32)

    kc_map = {}
    for b in range(B):
        al = int(actual_lens[b])
        for si in range(SC):
            s0 = si * 128
            if s0 >= al:
                break
            s1 = min(s0 + 128, al)
            lo = int(seg_id[b, s0])
            hi = int(seg_id[b, s1 - 1])
            for kc in range(lo // 128, hi // 128 + 1):
                kc_map.setdefault(kc, set()).add(si)
    kc_sis = tuple(
        (kc, tuple(sorted(kc_map[kc]))) for kc in sorted(kc_map)
    )

    nc2 = build_phase2(k_pad, kc_sis)
    import ml_dtypes

    xh_all = hidden.astype(ml_dtypes.bfloat16)
    xl_all = (hidden - xh_all.astype(np.float32)).astype(ml_dtypes.bfloat16)
    in_maps2 = [
        {"xh": xh_all[b], "xl": xl_all[b],
         "msid": np.ascontiguousarray(msid[b].reshape(SC, 128).T)}
        for b in range(B)
    ]
    global _last_in_maps1, _last_in_maps2
    _last_in_maps1 = in_maps1
    _last_in_maps2 = in_maps2
    res2 = _run_spmd_cached(("p2", k_pad, kc_sis), nc2, in_maps2)
    pooled_sum = np.stack([res2[b]["ps"][:K] for b in range(B)])  # [B,K,D]

    counts = np.zeros((B, K), dtype=np.float32)
    for b in range(B):
        v = valid[b] > 0
        ids = seg_id[b, v].astype(np.int64)
        if ids.size:
            cnt = np.bincount(ids, minlength=K)
            counts[b] = cnt[:K]
    counts = np.maximum(counts, np.float32(1.0))
    pooled = pooled_sum / counts[:, :, None]

    return (
        pooled.astype(np.float32),
        masked_probs.astype(np.float32),
        shortened,
        nb.astype(np.float32),
        actual_lens.astype(np.float32),
    )


# revision 14
# speedup vs baseline: 1.2695x; 1.2520x over previous
"""Trainium2 Bass kernel for nn_BoundaryPredictorMLP (segment-mean pooling MLP).

Sharding: pure data parallel — one batch sample per NeuronCore (B=8, 8 cores).

Pipeline (two device launches with a host step between — K is data-dependent):
  Launch 1 (device, fp32): G^T = (X @ W1)^T per core.  fp32 PE matmul keeps
    logit error ~1e-7 (boundary-threshold margin on this data is ~5e-5, so
    bf16-class matmuls would flip segment boundaries).
  Host: gelu (exact, erf-based) + logits + sigmoid via jax-CPU; boundary /
    segment-id bookkeeping in numpy mirroring the reference fp32 arithmetic.
  Launch 2 (device, fp32): segment-sum pooling as a one-hot matmul.  The
    one-hot matrix [S, K_pad] is built on device from masked seg-ids via
    iota + per-partition is_equal.  Only (s-chunk, k-chunk) pairs that can be
    non-zero on some core are emitted (segments are contiguous -> banded).
  Host: divide by counts, assemble the 5 reference outputs.
"""

import numpy as np

import concourse.bacc as bacc
import concourse.tile as tile
from concourse import mybir

F32 = mybir.dt.float32

B, S, D = 8, 2048, 768
NCORES = 8
SC = S // 128  # 16 s-chunks
DC = D // 128  # 6 d-chunks

_nc_cache = {}


def _run_spmd_cached(cache_key, nc, in_maps):
    """run_bass_via_pjrt with the jitted executable cached across calls.

    Mirrors concourse.bass2jax.run_bass_via_pjrt's multi-core path (shard_map
    over 8 devices, inputs concatenated on axis 0, donated zero output
    buffers) but keeps the compiled callable so repeat kernel() invocations
    skip retracing/recompiling.
    """
    import jax
    from jax.sharding import Mesh, PartitionSpec
    from jax.experimental.shard_map import shard_map
    from concourse import bass2jax, mybir as _mb

    n_cores = len(in_maps)
    entry = _nc_cache.get(("jit", cache_key))
    if entry is None:
        bass2jax.install_neuronx_cc_hook()
        in_names, out_names, out_avals = [], [], []
        partition_name = (
            nc.partition_id_tensor.name if nc.partition_id_tensor else None
        )
        for alloc in nc.m.functions[0].allocations:
            if not isinstance(alloc, _mb.MemoryLocationSet):
                continue
            name = alloc.memorylocations[0].name
            if alloc.kind == "ExternalInput":
                if name != partition_name:
                    in_names.append(name)
            elif alloc.kind == "ExternalOutput":
                out_names.append(name)
                out_avals.append(
                    jax.core.ShapedArray(
                        tuple(alloc.tensor_shape), _mb.dt.np(alloc.dtype)
                    )
                )
        n_params = len(in_names)
        n_outs = len(out_avals)
        all_in_names = list(in_names) + list(out_names)
        if partition_name is not None:
            all_in_names.append(partition_name)

        def _body(*args):
            operands = list(args)
            if partition_name is not None:
                operands.append(bass2jax.partition_id_tensor())
            outs = bass2jax._bass_exec_p.bind(
                *operands,
                out_avals=tuple(out_avals),
                in_names=tuple(all_in_names),
                out_names=tuple(out_names),
                lowering_input_output_aliases=(),
                sim_require_finite=True,
                sim_require_nnan=True,
                nc=nc,
            )
            return tuple(outs)

        devices = jax.devices()[:n_cores]
        mesh = Mesh(np.asarray(devices), ("core",))
        in_specs = (PartitionSpec("core"),) * (n_params + n_outs)
        out_specs = (PartitionSpec("core"),) * n_outs
        donate = tuple(range(n_params, n_params + n_outs))
        fn = jax.jit(
            shard_map(
                _body, mesh=mesh, in_specs=in_specs, out_specs=out_specs,
                check_rep=False,
            ),
            donate_argnums=donate,
            keep_unused=True,
        )
        entry = (fn, in_names, out_names, out_avals)
        _nc_cache[("jit", cache_key)] = entry

    fn, in_names, out_names, out_avals = entry
    concat_in = [
        np.concatenate([np.asarray(m[name]) for m in in_maps], axis=0)
        for name in in_names
    ]
    concat_zeros = [
        np.zeros((n_cores * a.shape[0], *a.shape[1:]), a.dtype) for a in out_avals
    ]
    out_arrs = fn(*concat_in, *concat_zeros)
    return [
        {
            name: np.asarray(out_arrs[i]).reshape(n_cores, *out_avals[i].shape)[c]
            for i, name in enumerate(out_names)
        }
        for c in range(n_cores)
    ]


def build_phase1():
    """Device MLP: logits[1,S] = gelu((X@W1)^T + b1) dot W2.

    The big GEMM runs in float32r (full-rate on PE); gelu+bias on ScalarE
    straight out of PSUM; the W2 contraction is a skinny fp32 matmul.  Rows
    with |logit| near the 0 threshold are exactly recomputed on the host, so
    f32r/LUT error cannot flip a boundary decision.
    """
    if "p1" in _nc_cache:
        return _nc_cache["p1"]
    F32R = mybir.dt.float32r
    nc = bacc.Bacc()
    xt = nc.declare_dram_parameter("xt", [D, S], F32R, isOutput=False)
    w1 = nc.declare_dram_parameter("w1", [D, D], F32R, isOutput=False)
    b1p = nc.declare_dram_parameter("b1", [128, DC], F32, isOutput=False)
    w2 = nc.declare_dram_parameter("w2", [128, DC], F32R, isOutput=False)
    lg = nc.declare_dram_parameter("lg", [1, S], F32, isOutput=True)

    with tile.TileContext(nc) as tc:
        with (
            tc.tile_pool(name="sb", bufs=1) as sb,
            tc.tile_pool(name="ps", bufs=4, space="PSUM") as ps,
            tc.tile_pool(name="psl", bufs=2, space="PSUM") as psl,
        ):
            w2t = sb.tile([128, DC], F32R, tag="w2t")
            nc.sync.dma_start(out=w2t, in_=w2[:, :])
            b1t = sb.tile([128, DC], F32, tag="b1t")
            nc.sync.dma_start(out=b1t, in_=b1p[:, :])
            # W1 split by output-column chunk: the first PSUM group only needs
            # mc=0 (384 KB), not the whole 2.25 MB
            w1t = sb.tile([128, DC, D], F32R, tag="w1t")
            w1_r = w1[:, :].rearrange("(kc p) (mc m) -> p kc mc m", p=128, m=128)
            for mc in range(DC):
                nc.sync.dma_start(
                    out=w1t[:, :, mc * 128 : (mc + 1) * 128], in_=w1_r[:, :, mc, :]
                )
            NS = S // 512
            xtt = sb.tile([128, DC, NS, 512], F32R, tag="xtt")
            xt_r = xt[:, :].rearrange("(kc p) (n s) -> p kc n s", p=128, s=512)
            for n in range(NS):
                # per-s-block DMA so the first matmul group only waits for
                # W1[mc0] + one 1.6 MB slice instead of the full transfer
                nc.scalar.dma_start(out=xtt[:, :, n, :], in_=xt_r[:, :, n, :])
            ht = sb.tile([128, DC, S], F32R, tag="ht")
            lt = sb.tile([1, S], F32, tag="lt")
            def emit_mlp2(n):
                pl = psl.tile([1, 512], F32, tag="pl")
                for mc in range(DC):
                    nc.tensor.matmul(
                        pl,
                        w2t[:, mc : mc + 1],
                        ht[:, mc, n * 512 : (n + 1) * 512],
                        start=(mc == 0),
                        stop=(mc == DC - 1),
                    )
                nc.vector.tensor_copy(lt[:, n * 512 : (n + 1) * 512], pl)

            for n in range(NS):
                for mc in range(DC):
                    pt = ps.tile([128, 512], F32, tag="pt")
                    for kc in range(DC):
                        nc.tensor.matmul(
                            pt,
                            w1t[:, kc, mc * 128 : (mc + 1) * 128],
                            xtt[:, kc, n, :],
                            start=(kc == 0),
                            stop=(kc == DC - 1),
                        )
                    nc.scalar.activation(
                        out=ht[:, mc, n * 512 : (n + 1) * 512],
                        in_=pt,
                        func=mybir.ActivationFunctionType.Gelu,
                        bias=b1t[:, mc : mc + 1],
                        scale=1.0,
                    )
                if n >= 1:
                    emit_mlp2(n - 1)
            emit_mlp2(NS - 1)
            nc.sync.dma_start(out=lg[:, :], in_=lt)
    nc.finalize()
    _nc_cache["p1"] = nc
    return nc


def build_phase2(k_pad, kc_sis):
    """Pooled_sum [K_pad, D] = onehot[S,K_pad]^T @ X[S,D], banded over kc_sis.

    kc_sis: tuple of (kc, (si, si, ...)) — which s-chunks feed each k-chunk.
    """
    key = ("p2", k_pad, kc_sis)
    if key in _nc_cache:
        return _nc_cache[key]
    BF16 = mybir.dt.bfloat16
    nc = bacc.Bacc()
    xhp = nc.declare_dram_parameter("xh", [S, D], BF16, isOutput=False)
    xlp = nc.declare_dram_parameter("xl", [S, D], BF16, isOutput=False)
    msid = nc.declare_dram_parameter("msid", [128, SC], F32, isOutput=False)
    psum_out = nc.declare_dram_parameter("ps", [k_pad, D], F32, isOutput=True)

    # per-s-chunk k-band: contiguous kc cover of every pair this si is in
    si_band = {}
    for kc, sis in kc_sis:
        for si in sis:
            lo, hi = si_band.get(si, (kc, kc))
            si_band[si] = (min(lo, kc), max(hi, kc))
    bw = max(hi - lo + 1 for lo, hi in si_band.values())  # chunks

    with tile.TileContext(nc) as tc:
        with (
            tc.tile_pool(name="sb", bufs=1) as sb,
            tc.tile_pool(name="stg", bufs=4) as stg,
            tc.tile_pool(name="ps", bufs=4, space="PSUM") as ps,
        ):
            ms = sb.tile([128, SC], F32, tag="ms")
            nc.sync.dma_start(out=ms, in_=msid[:, :])
            xh = sb.tile([128, SC, D], BF16, tag="xh")
            xl = sb.tile([128, SC, D], BF16, tag="xl")
            xh_r = xhp[:, :].rearrange("(g si p) d -> p g si d", p=128, si=4)
            xl_r = xlp[:, :].rearrange("(g si p) d -> p g si d", p=128, si=4)
            for g in range(SC // 4):
                nc.scalar.dma_start(out=xh[:, 4 * g : 4 * g + 4, :], in_=xh_r[:, g])
                nc.scalar.dma_start(out=xl[:, 4 * g : 4 * g + 4, :], in_=xl_r[:, g])
            it = sb.tile([128, k_pad], F32, tag="it")
            nc.gpsimd.iota(
                it,
                pattern=[[1, k_pad]],
                base=0,
                channel_multiplier=0,
                allow_small_or_imprecise_dtypes=True,
            )
            oh = sb.tile([128, SC, bw * 128], BF16, tag="oh")
            for si, (lo, hi) in sorted(si_band.items()):
                w = (hi - lo + 1) * 128
                nc.vector.tensor_scalar(
                    out=oh[:, si, 0:w],
                    in0=it[:, lo * 128 : lo * 128 + w],
                    scalar1=ms[:, si : si + 1],
                    scalar2=None,
                    op0=mybir.AluOpType.is_equal,
                )
            for kc, sis in kc_sis:
                pa = ps.tile([128, 512], F32, tag="pa")
                pb = ps.tile([128, 256], F32, tag="pb")
                n_si = len(sis)
                for j, si in enumerate(sis):
                    st, sp = (j == 0), (j == n_si - 1)
                    off = (kc - si_band[si][0]) * 128
                    lhs = oh[:, si, off : off + 128]
                    nc.tensor.matmul(pa, lhs, xh[:, si, 0:512], start=st, stop=False)
                    nc.tensor.matmul(pa, lhs, xl[:, si, 0:512], start=False, stop=sp)
                    nc.tensor.matmul(pb, lhs, xh[:, si, 512:768], start=st, stop=False)
                    nc.tensor.matmul(pb, lhs, xl[:, si, 512:768], start=False, stop=sp)
                oa = stg.tile([128, D], F32, tag="oa")
                nc.vector.tensor_copy(oa[:, 0:512], pa)
                nc.vector.tensor_copy(oa[:, 512:768], pb)
                nc.sync.dma_start(
                    out=psum_out[kc * 128 : (kc + 1) * 128, :], in_=oa
                )
    nc.finalize()
    _nc_cache[key] = nc
    return nc


# |logit| band inside which the host exactly recomputes the MLP row.  Device
# logit error (f32r GEMM + gelu LUT) is measured at ~1e-4 on this workload;
# 1e-2 gives a ~100x safety margin at ~2% of rows recomputed.
TAU = np.float32(1e-2)


def _exact_rows(hidden, W1, b1, W2, b2, bb, ss):
    """Exact fp32 reference MLP for selected (batch, seq) rows."""
    import math

    from scipy.special import erf

    Xrows = hidden[bb, ss, :].astype(np.float32)  # [R, D]
    Grows = (Xrows @ W1 + b1).astype(np.float64)
    hrows = (Grows * 0.5 * (1.0 + erf(Grows / math.sqrt(2.0)))).astype(np.float32)
    return (hrows @ W2 + b2).astype(np.float32)


def _rne11(x):
    """Round fp32 to float32r's grid: round-to-nearest-even, 11 mantissa bits.

    Bit-identical to the device's fp32->f32r cast (verified on HW), so
    pre-rounding on the host and DMAing without cast preserves numerics.
    """
    b = np.ascontiguousarray(x, np.float32).view(np.uint32).astype(np.uint64)
    add = np.uint64((1 << 11) - 1)
    lsb = (b >> np.uint64(12)) & np.uint64(1)
    out = ((b + add + lsb) >> np.uint64(12) << np.uint64(12)).astype(np.uint32)
    return out.view(np.float32)


def kernel(hidden, lengths, W1, b1, W2, b2):
    hidden = np.ascontiguousarray(hidden, dtype=np.float32)
    lengths = np.asarray(lengths, dtype=np.float32)
    W1 = np.ascontiguousarray(W1, dtype=np.float32)
    b1 = np.asarray(b1, dtype=np.float32)
    W2 = np.asarray(W2, dtype=np.float32)
    b2 = np.asarray(b2, dtype=np.float32)

    # ---------------- Phase 1: logits on device ----------------
    nc1 = build_phase1()
    w1r = _rne11(W1)
    w2c = np.ascontiguousarray(_rne11(W2).reshape(DC, 128).T)
    b1c = np.ascontiguousarray(b1.reshape(DC, 128).T)
    xtr = _rne11(np.ascontiguousarray(hidden.transpose(0, 2, 1)))
    in_maps1 = [
        {"xt": xtr[b], "w1": w1r, "b1": b1c, "w2": w2c} for b in range(B)
    ]
    res1 = _run_spmd_cached("p1", nc1, in_maps1)
    logits = np.stack([res1[b]["lg"].reshape(S) for b in range(B)])  # [B,S]
    logits = (logits + b2).astype(np.float32)

    # exact host recompute of rows near the decision threshold
    band = np.abs(logits) < TAU
    if band.any():
        bb, ss = np.nonzero(band)
        logits[bb, ss] = _exact_rows(hidden, W1, b1, W2, b2, bb, ss)

    with np.errstate(over="ignore"):
        probs = (np.float32(1.0) / (np.float32(1.0) + np.exp(-logits))).astype(
            np.float32
        )

    actual_lens = (lengths * np.float32(S)).astype(np.int32)  # [B]
    sidx = np.arange(S, dtype=np.int64)
    valid = (sidx[None, :] < actual_lens[:, None]).astype(np.float32)  # [B,S]
    soft = probs * valid
    hard = (probs > np.float32(0.5)).astype(np.float32) * valid
    last_valid = np.clip(actual_lens - 1, 0, S - 1)
    bi = np.arange(B)
    soft[bi, last_valid] = np.float32(1.0)
    hard[bi, last_valid] = np.float32(1.0)
    hard_b = (hard - soft) + soft  # exact == hard in fp32; mirrors reference
    K = int(np.max(np.sum(hard_b, axis=1)))
    seg_id = np.cumsum(hard_b, axis=1, dtype=np.float32) - hard_b  # [B,S]
    nb = np.sum(hard_b * valid, axis=1)  # [B] f32

    masked_probs = probs * valid

    max_segments = max(K, 1)
    full = nb >= max_segments - 1
    partial = (nb > 0) & (nb < max_segments - 1)
    shortened = np.where(
        full,
        np.float32(1.0),
        np.where(partial, (nb + np.float32(1.0)) / np.float32(max_segments),
                 np.float32(0.0)),
    ).astype(np.float32)

    # ---------------- Phase 2: banded one-hot segment-sum ----------------
    k_pad = ((K + 127) // 128) * 128
    msid = np.where(valid > 0, seg_id, np.float32(-1.0)).astype(np.float32)

    kc_map = {}
    for b in range(B):
        al = int(actual_lens[b])
        for si in range(SC):
            s0 = si * 128
            if s0 >= al:
                break
            s1 = min(s0 + 128, al)
            lo = int(seg_id[b, s0])
            hi = int(seg_id[b, s1 - 1])
            for kc in range(lo // 128, hi // 128 + 1):
                kc_map.setdefault(kc, set()).add(si)
    kc_sis = tuple(
        (kc, tuple(sorted(kc_map[kc]))) for kc in sorted(kc_map)
    )

    nc2 = build_phase2(k_pad, kc_sis)
    import ml_dtypes

    xh_all = hidden.astype(ml_dtypes.bfloat16)
    xl_all = (hidden - xh_all.astype(np.float32)).astype(ml_dtypes.bfloat16)
    in_maps2 = [
        {"xh": xh_all[b], "xl": xl_all[b],
         "msid": np.ascontiguousarray(msid[b].reshape(SC, 128).T)}
        for b in range(B)
    ]
    global _last_in_maps1, _last_in_maps2
    _last_in_maps1 = in_maps1
    _last_in_maps2 = in_maps2
    res2 = _run_spmd_cached(("p2", k_pad, kc_sis), nc2, in_maps2)
    pooled_sum = np.stack([res2[b]["ps"][:K] for b in range(B)])  # [B,K,D]

    counts = np.zeros((B, K), dtype=np.float32)
    for b in range(B):
        v = valid[b] > 0
        ids = seg_id[b, v].astype(np.int64)
        if ids.size:
            cnt = np.bincount(ids, minlength=K)
            counts[b] = cnt[:K]
    counts = np.maximum(counts, np.float32(1.0))
    pooled = pooled_sum / counts[:, :, None]

    return (
        pooled.astype(np.float32),
        masked_probs.astype(np.float32),
        shortened,
        nb.astype(np.float32),
        actual_lens.astype(np.float32),
    )


# revision 16
# speedup vs baseline: 1.3253x; 1.0440x over previous
"""Trainium2 Bass kernel for nn_BoundaryPredictorMLP (segment-mean pooling MLP).

Sharding: pure data parallel — one batch sample per NeuronCore (B=8, 8 cores).

Pipeline (two device launches with a host step between — K is data-dependent):
  Launch 1 (device, fp32): G^T = (X @ W1)^T per core.  fp32 PE matmul keeps
    logit error ~1e-7 (boundary-threshold margin on this data is ~5e-5, so
    bf16-class matmuls would flip segment boundaries).
  Host: gelu (exact, erf-based) + logits + sigmoid via jax-CPU; boundary /
    segment-id bookkeeping in numpy mirroring the reference fp32 arithmetic.
  Launch 2 (device, fp32): segment-sum pooling as a one-hot matmul.  The
    one-hot matrix [S, K_pad] is built on device from masked seg-ids via
    iota + per-partition is_equal.  Only (s-chunk, k-chunk) pairs that can be
    non-zero on some core are emitted (segments are contiguous -> banded).
  Host: divide by counts, assemble the 5 reference outputs.
"""

import numpy as np

import concourse.bacc as bacc
import concourse.tile as tile
from concourse import mybir

F32 = mybir.dt.float32

B, S, D = 8, 2048, 768
NCORES = 8
SC = S // 128  # 16 s-chunks
DC = D // 128  # 6 d-chunks

_nc_cache = {}


def _run_spmd_cached(cache_key, nc, in_maps):
    """run_bass_via_pjrt with the jitted executable cached across calls.

    Mirrors concourse.bass2jax.run_bass_via_pjrt's multi-core path (shard_map
    over 8 devices, inputs concatenated on axis 0, donated zero output
    buffers) but keeps the compiled callable so repeat kernel() invocations
    skip retracing/recompiling.
    """
    import jax
    from jax.sharding import Mesh, PartitionSpec
    from jax.experimental.shard_map import shard_map
    from concourse import bass2jax, mybir as _mb

    n_cores = len(in_maps)
    entry = _nc_cache.get(("jit", cache_key))
    if entry is None:
        bass2jax.install_neuronx_cc_hook()
        in_names, out_names, out_avals = [], [], []
        partition_name = (
            nc.partition_id_tensor.name if nc.partition_id_tensor else None
        )
        for alloc in nc.m.functions[0].allocations:
            if not isinstance(alloc, _mb.MemoryLocationSet):
                continue
            name = alloc.memorylocations[0].name
            if alloc.kind == "ExternalInput":
                if name != partition_name:
                    in_names.append(name)
            elif alloc.kind == "ExternalOutput":
                out_names.append(name)
                out_avals.append(
                    jax.core.ShapedArray(
                        tuple(alloc.tensor_shape), _mb.dt.np(alloc.dtype)
                    )
                )
        n_params = len(in_names)
        n_outs = len(out_avals)
        all_in_names = list(in_names) + list(out_names)
        if partition_name is not None:
            all_in_names.append(partition_name)

        def _body(*args):
            operands = list(args)
            if partition_name is not None:
                operands.append(bass2jax.partition_id_tensor())
            outs = bass2jax._bass_exec_p.bind(
                *operands,
                out_avals=tuple(out_avals),
                in_names=tuple(all_in_names),
                out_names=tuple(out_names),
                lowering_input_output_aliases=(),
                sim_require_finite=True,
                sim_require_nnan=True,
                nc=nc,
            )
            return tuple(outs)

        devices = jax.devices()[:n_cores]
        mesh = Mesh(np.asarray(devices), ("core",))
        in_specs = (PartitionSpec("core"),) * (n_params + n_outs)
        out_specs = (PartitionSpec("core"),) * n_outs
        donate = tuple(range(n_params, n_params + n_outs))
        fn = jax.jit(
            shard_map(
                _body, mesh=mesh, in_specs=in_specs, out_specs=out_specs,
                check_rep=False,
            ),
            donate_argnums=donate,
            keep_unused=True,
        )
        entry = (fn, in_names, out_names, out_avals)
        _nc_cache[("jit", cache_key)] = entry

    fn, in_names, out_names, out_avals = entry
    concat_in = [
        np.concatenate([np.asarray(m[name]) for m in in_maps], axis=0)
        for name in in_names
    ]
    concat_zeros = [
        np.zeros((n_cores * a.shape[0], *a.shape[1:]), a.dtype) for a in out_avals
    ]
    out_arrs = fn(*concat_in, *concat_zeros)
    return [
        {
            name: np.asarray(out_arrs[i]).reshape(n_cores, *out_avals[i].shape)[c]
            for i, name in enumerate(out_names)
        }
        for c in range(n_cores)
    ]


def build_phase1():
    """Device MLP: logits[1,S] = gelu((X@W1)^T + b1) dot W2.

    The big GEMM runs in float32r (full-rate on PE); gelu+bias on ScalarE
    straight out of PSUM; the W2 contraction is a skinny fp32 matmul.  Rows
    with |logit| near the 0 threshold are exactly recomputed on the host, so
    f32r/LUT error cannot flip a boundary decision.
    """
    if "p1" in _nc_cache:
        return _nc_cache["p1"]
    F32R = mybir.dt.float32r
    nc = bacc.Bacc()
    xt = nc.declare_dram_parameter("xt", [D, S], F32R, isOutput=False)
    w1 = nc.declare_dram_parameter("w1", [D, D], F32R, isOutput=False)
    b1p = nc.declare_dram_parameter("b1", [128, DC], F32, isOutput=False)
    w2 = nc.declare_dram_parameter("w2", [128, DC], F32R, isOutput=False)
    lg = nc.declare_dram_parameter("lg", [1, S], F32, isOutput=True)

    with tile.TileContext(nc) as tc:
        with (
            tc.tile_pool(name="sb", bufs=1) as sb,
            tc.tile_pool(name="ps", bufs=4, space="PSUM") as ps,
            tc.tile_pool(name="psl", bufs=2, space="PSUM") as psl,
        ):
            w2t = sb.tile([128, DC], F32R, tag="w2t")
            nc.sync.dma_start(out=w2t, in_=w2[:, :])
            b1t = sb.tile([128, DC], F32, tag="b1t")
            nc.sync.dma_start(out=b1t, in_=b1p[:, :])
            # W1 split by output-column chunk: the first PSUM group only needs
            # mc=0 (384 KB), not the whole 2.25 MB
            w1t = sb.tile([128, DC, D], F32R, tag="w1t")
            w1_r = w1[:, :].rearrange("(kc p) (mc m) -> p kc mc m", p=128, m=128)
            for mc in range(DC):
                nc.sync.dma_start(
                    out=w1t[:, :, mc * 128 : (mc + 1) * 128], in_=w1_r[:, :, mc, :]
                )
            NS = S // 512
            xtt = sb.tile([128, DC, NS, 512], F32R, tag="xtt")
            xt_r = xt[:, :].rearrange("(kc p) (n s) -> p kc n s", p=128, s=512)
            for n in range(NS):
                # per-s-block DMA so the first matmul group only waits for
                # W1[mc0] + one slice instead of the full transfer; the first
                # slice is further split by kc-halves to start PE earlier
                if n == 0:
                    nc.scalar.dma_start(
                        out=xtt[:, 0:3, n, :], in_=xt_r[:, 0:3, n, :]
                    )
                    nc.scalar.dma_start(
                        out=xtt[:, 3:6, n, :], in_=xt_r[:, 3:6, n, :]
                    )
                else:
                    nc.scalar.dma_start(out=xtt[:, :, n, :], in_=xt_r[:, :, n, :])
            ht = sb.tile([128, DC, S], F32R, tag="ht")
            lt = sb.tile([1, S], F32, tag="lt")
            def emit_mlp2(n):
                pl = psl.tile([1, 512], F32, tag="pl")
                for mc in range(DC):
                    nc.tensor.matmul(
                        pl,
                        w2t[:, mc : mc + 1],
                        ht[:, mc, n * 512 : (n + 1) * 512],
                        start=(mc == 0),
                        stop=(mc == DC - 1),
                    )
                nc.vector.tensor_copy(lt[:, n * 512 : (n + 1) * 512], pl)

            for n in range(NS):
                for mc in range(DC):
                    pt = ps.tile([128, 512], F32, tag="pt")
                    for kc in range(DC):
                        nc.tensor.matmul(
                            pt,
                            w1t[:, kc, mc * 128 : (mc + 1) * 128],
                            xtt[:, kc, n, :],
                            start=(kc == 0),
                            stop=(kc == DC - 1),
                        )
                    nc.scalar.activation(
                        out=ht[:, mc, n * 512 : (n + 1) * 512],
                        in_=pt,
                        func=mybir.ActivationFunctionType.Gelu,
                        bias=b1t[:, mc : mc + 1],
                        scale=1.0,
                    )
                if n >= 1:
                    emit_mlp2(n - 1)
            emit_mlp2(NS - 1)
            nc.sync.dma_start(out=lg[:, :], in_=lt)
    nc.finalize()
    _nc_cache["p1"] = nc
    return nc


def build_phase2(k_pad, kc_sis):
    """Pooled_sum [K_pad, D] = onehot[S,K_pad]^T @ X[S,D], banded over kc_sis.

    kc_sis: tuple of (kc, (si, si, ...)) — which s-chunks feed each k-chunk.
    """
    key = ("p2", k_pad, kc_sis)
    if key in _nc_cache:
        return _nc_cache[key]
    BF16 = mybir.dt.bfloat16
    nc = bacc.Bacc()
    xhp = nc.declare_dram_parameter("xh", [S, D], BF16, isOutput=False)
    xlp = nc.declare_dram_parameter("xl", [S, D], BF16, isOutput=False)
    msid = nc.declare_dram_parameter("msid", [128, SC], F32, isOutput=False)
    psum_out = nc.declare_dram_parameter("ps", [k_pad, D], F32, isOutput=True)

    # per-s-chunk k-band: contiguous kc cover of every pair this si is in
    si_band = {}
    for kc, sis in kc_sis:
        for si in sis:
            lo, hi = si_band.get(si, (kc, kc))
            si_band[si] = (min(lo, kc), max(hi, kc))
    bw = max(hi - lo + 1 for lo, hi in si_band.values())  # chunks

    with tile.TileContext(nc) as tc:
        with (
            tc.tile_pool(name="sb", bufs=1) as sb,
            tc.tile_pool(name="stg", bufs=4) as stg,
            tc.tile_pool(name="ps", bufs=4, space="PSUM") as ps,
        ):
            ms = sb.tile([128, SC], F32, tag="ms")
            nc.sync.dma_start(out=ms, in_=msid[:, :])
            xh = sb.tile([128, SC, D], BF16, tag="xh")
            xl = sb.tile([128, SC, D], BF16, tag="xl")
            xh_r = xhp[:, :].rearrange("(g si p) d -> p g si d", p=128, si=4)
            xl_r = xlp[:, :].rearrange("(g si p) d -> p g si d", p=128, si=4)
            for g in range(SC // 4):
                if g == 0:
                    xh_r2 = xhp[:, :].rearrange("(h si p) d -> p h si d", p=128, si=2)
                    xl_r2 = xlp[:, :].rearrange("(h si p) d -> p h si d", p=128, si=2)
                    for h in range(2):
                        nc.scalar.dma_start(
                            out=xh[:, 2 * h : 2 * h + 2, :], in_=xh_r2[:, h]
                        )
                        nc.scalar.dma_start(
                            out=xl[:, 2 * h : 2 * h + 2, :], in_=xl_r2[:, h]
                        )
                else:
                    nc.scalar.dma_start(out=xh[:, 4 * g : 4 * g + 4, :], in_=xh_r[:, g])
                    nc.scalar.dma_start(out=xl[:, 4 * g : 4 * g + 4, :], in_=xl_r[:, g])
            it = sb.tile([128, k_pad], F32, tag="it")
            nc.gpsimd.iota(
                it,
                pattern=[[1, k_pad]],
                base=0,
                channel_multiplier=0,
                allow_small_or_imprecise_dtypes=True,
            )
            oh = sb.tile([128, SC, bw * 128], BF16, tag="oh")
            for si, (lo, hi) in sorted(si_band.items()):
                w = (hi - lo + 1) * 128
                nc.vector.tensor_scalar(
                    out=oh[:, si, 0:w],
                    in0=it[:, lo * 128 : lo * 128 + w],
                    scalar1=ms[:, si : si + 1],
                    scalar2=None,
                    op0=mybir.AluOpType.is_equal,
                )
            for kc, sis in kc_sis:
                pa = ps.tile([128, 512], F32, tag="pa")
                pb = ps.tile([128, 256], F32, tag="pb")
                n_si = len(sis)
                for j, si in enumerate(sis):
                    st, sp = (j == 0), (j == n_si - 1)
                    off = (kc - si_band[si][0]) * 128
                    lhs = oh[:, si, off : off + 128]
                    nc.tensor.matmul(pa, lhs, xh[:, si, 0:512], start=st, stop=False)
                    nc.tensor.matmul(pa, lhs, xl[:, si, 0:512], start=False, stop=sp)
                    nc.tensor.matmul(pb, lhs, xh[:, si, 512:768], start=st, stop=False)
                    nc.tensor.matmul(pb, lhs, xl[:, si, 512:768], start=False, stop=sp)
                oa = stg.tile([128, D], F32, tag="oa")
                nc.vector.tensor_copy(oa[:, 0:512], pa)
                nc.vector.tensor_copy(oa[:, 512:768], pb)
                nc.sync.dma_start(
                    out=psum_out[kc * 128 : (kc + 1) * 128, :], in_=oa
                )
    nc.finalize()
    _nc_cache[key] = nc
    return nc


# |logit| band inside which the host exactly recomputes the MLP row.  Device
# logit error (f32r GEMM + gelu LUT) is measured at ~1e-4 on this workload;
# 1e-2 gives a ~100x safety margin at ~2% of rows recomputed.
TAU = np.float32(1e-2)


def _exact_rows(hidden, W1, b1, W2, b2, bb, ss):
    """Exact fp32 reference MLP for selected (batch, seq) rows."""
    import math

    from scipy.special import erf

    Xrows = hidden[bb, ss, :].astype(np.float32)  # [R, D]
    Grows = (Xrows @ W1 + b1).astype(np.float64)
    hrows = (Grows * 0.5 * (1.0 + erf(Grows / math.sqrt(2.0)))).astype(np.float32)
    return (hrows @ W2 + b2).astype(np.float32)


def _rne11(x):
    """Round fp32 to float32r's grid: round-to-nearest-even, 11 mantissa bits.

    Bit-identical to the device's fp32->f32r cast (verified on HW), so
    pre-rounding on the host and DMAing without cast preserves numerics.
    """
    b = np.ascontiguousarray(x, np.float32).view(np.uint32).astype(np.uint64)
    add = np.uint64((1 << 11) - 1)
    lsb = (b >> np.uint64(12)) & np.uint64(1)
    out = ((b + add + lsb) >> np.uint64(12) << np.uint64(12)).astype(np.uint32)
    return out.view(np.float32)


def kernel(hidden, lengths, W1, b1, W2, b2):
    hidden = np.ascontiguousarray(hidden, dtype=np.float32)
    lengths = np.asarray(lengths, dtype=np.float32)
    W1 = np.ascontiguousarray(W1, dtype=np.float32)
    b1 = np.asarray(b1, dtype=np.float32)
    W2 = np.asarray(W2, dtype=np.float32)
    b2 = np.asarray(b2, dtype=np.float32)

    # ---------------- Phase 1: logits on device ----------------
    nc1 = build_phase1()
    w1r = _rne11(W1)
    w2c = np.ascontiguousarray(_rne11(W2).reshape(DC, 128).T)
    b1c = np.ascontiguousarray(b1.reshape(DC, 128).T)
    xtr = _rne11(np.ascontiguousarray(hidden.transpose(0, 2, 1)))
    in_maps1 = [
        {"xt": xtr[b], "w1": w1r, "b1": b1c, "w2": w2c} for b in range(B)
    ]
    res1 = _run_spmd_cached("p1", nc1, in_maps1)
    logits = np.stack([res1[b]["lg"].reshape(S) for b in range(B)])  # [B,S]
    logits = (logits + b2).astype(np.float32)

    # exact host recompute of rows near the decision threshold
    band = np.abs(logits) < TAU
    if band.any():
        bb, ss = np.nonzero(band)
        logits[bb, ss] = _exact_rows(hidden, W1, b1, W2, b2, bb, ss)

    with np.errstate(over="ignore"):
        probs = (np.float32(1.0) / (np.float32(1.0) + np.exp(-logits))).astype(
            np.float32
        )

    actual_lens = (lengths * np.float32(S)).astype(np.int32)  # [B]
    sidx = np.arange(S, dtype=np.int64)
    valid = (sidx[None, :] < actual_lens[:, None]).astype(np.float32)  # [B,S]
    soft = probs * valid
    hard = (probs > np.float32(0.5)).astype(np.float32) * valid
    last_valid = np.clip(actual_lens - 1, 0, S - 1)
    bi = np.arange(B)
    soft[bi, last_valid] = np.float32(1.0)
    hard[bi, last_valid] = np.float32(1.0)
    hard_b = (hard - soft) + soft  # exact == hard in fp32; mirrors reference
    K = int(np.max(np.sum(hard_b, axis=1)))
    seg_id = np.cumsum(hard_b, axis=1, dtype=np.float32) - hard_b  # [B,S]
    nb = np.sum(hard_b * valid, axis=1)  # [B] f32

    masked_probs = probs * valid

    max_segments = max(K, 1)
    full = nb >= max_segments - 1
    partial = (nb > 0) & (nb < max_segments - 1)
    shortened = np.where(
        full,
        np.float32(1.0),
        np.where(partial, (nb + np.float32(1.0)) / np.float32(max_segments),
                 np.float32(0.0)),
    ).astype(np.float32)

    # ---------------- Phase 2: banded one-hot segment-sum ----------------
    k_pad = ((K + 127) // 128) * 128
    msid = np.where(valid > 0, seg_id, np.float32(-1.0)).astype(np.float32)

    kc_map = {}
    for b in range(B):
        al = int(actual_lens[b])
        for si in range(SC):
            s0 = si * 128
            if s0 >= al:
                break
            s1 = min(s0 + 128, al)
            lo = int(seg_id[b, s0])
            hi = int(seg_id[b, s1 - 1])
            for kc in range(lo // 128, hi // 128 + 1):
                kc_map.setdefault(kc, set()).add(si)
    kc_sis = tuple(
        (kc, tuple(sorted(kc_map[kc]))) for kc in sorted(kc_map)
    )

    nc2 = build_phase2(k_pad, kc_sis)
    import ml_dtypes

    xh_all = hidden.astype(ml_dtypes.bfloat16)
    xl_all = (hidden - xh_all.astype(np.float32)).astype(ml_dtypes.bfloat16)
    in_maps2 = [
        {"xh": xh_all[b], "xl": xl_all[b],
         "msid": np.ascontiguousarray(msid[b].reshape(SC, 128).T)}
        for b in range(B)
    ]
    global _last_in_maps1, _last_in_maps2
    _last_in_maps1 = in_maps1
    _last_in_maps2 = in_maps2
    res2 = _run_spmd_cached(("p2", k_pad, kc_sis), nc2, in_maps2)
    pooled_sum = np.stack([res2[b]["ps"][:K] for b in range(B)])  # [B,K,D]

    counts = np.zeros((B, K), dtype=np.float32)
    for b in range(B):
        v = valid[b] > 0
        ids = seg_id[b, v].astype(np.int64)
        if ids.size:
            cnt = np.bincount(ids, minlength=K)
            counts[b] = cnt[:K]
    counts = np.maximum(counts, np.float32(1.0))
    pooled = pooled_sum / counts[:, :, None]

    return (
        pooled.astype(np.float32),
        masked_probs.astype(np.float32),
        shortened,
        nb.astype(np.float32),
        actual_lens.astype(np.float32),
    )


# revision 18
# speedup vs baseline: 1.3486x; 1.0176x over previous
"""Trainium2 Bass kernel for nn_BoundaryPredictorMLP (segment-mean pooling MLP).

Sharding: pure data parallel — one batch sample per NeuronCore (B=8, 8 cores).

Pipeline (two device launches with a host step between — the number of
segments K is data-dependent, so the pooling kernel's shapes/band structure
are only known after the MLP):

  Launch 1 (device): logits = gelu((X @ W1)^T + b1) . W2 per core.  The big
    GEMM runs in float32r (full-rate 1 cyc/row on PE vs fp32's 4; f32r = fp32
    rounded to 11 mantissa bits, pre-rounded on host so plain HWDGE DMAs
    work); gelu+bias on ScalarE straight out of PSUM; W2 contraction is a
    skinny f32r matmul software-pipelined one s-block behind the GEMM.
  Host: sigmoid / boundary extraction in numpy mirroring reference fp32
    arithmetic.  Rows with |logit| < TAU are recomputed exactly (fp32 BLAS +
    fp64 erf-gelu), so f32r/gelu-LUT error (~2e-4, TAU/err ~ 50x margin)
    cannot flip a probs>0.5 boundary decision — segment structure is exact.
  Launch 2 (device): segment-sum pooling as a one-hot matmul.  X is split
    hi/lo into two bf16 halves on host (exact products with the 0/1 one-hot,
    fp32 PSUM accumulate -> ~1e-6 pooled error at bf16 speed).  The one-hot
    [S, K_pad] is built on device from masked seg-ids via iota +
    per-partition is_equal, only over each s-chunk's reachable k-band, and
    only (s-chunk, k-chunk) matmuls that can be non-zero on some core are
    emitted (segments are contiguous -> banded).
  Host: divide by counts, assemble the 5 reference outputs.
"""

import numpy as np

import concourse.bacc as bacc
import concourse.tile as tile
from concourse import mybir

F32 = mybir.dt.float32

B, S, D = 8, 2048, 768
NCORES = 8
SC = S // 128  # 16 s-chunks
DC = D // 128  # 6 d-chunks

_nc_cache = {}


def _run_spmd_cached(cache_key, nc, in_maps):
    """run_bass_via_pjrt with the jitted executable cached across calls.

    Mirrors concourse.bass2jax.run_bass_via_pjrt's multi-core path (shard_map
    over 8 devices, inputs concatenated on axis 0, donated zero output
    buffers) but keeps the compiled callable so repeat kernel() invocations
    skip retracing/recompiling.
    """
    import jax
    from jax.sharding import Mesh, PartitionSpec
    from jax.experimental.shard_map import shard_map
    from concourse import bass2jax, mybir as _mb

    n_cores = len(in_maps)
    entry = _nc_cache.get(("jit", cache_key))
    if entry is None:
        bass2jax.install_neuronx_cc_hook()
        in_names, out_names, out_avals = [], [], []
        partition_name = (
            nc.partition_id_tensor.name if nc.partition_id_tensor else None
        )
        for alloc in nc.m.functions[0].allocations:
            if not isinstance(alloc, _mb.MemoryLocationSet):
                continue
            name = alloc.memorylocations[0].name
            if alloc.kind == "ExternalInput":
                if name != partition_name:
                    in_names.append(name)
            elif alloc.kind == "ExternalOutput":
                out_names.append(name)
                out_avals.append(
                    jax.core.ShapedArray(
                        tuple(alloc.tensor_shape), _mb.dt.np(alloc.dtype)
                    )
                )
        n_params = len(in_names)
        n_outs = len(out_avals)
        all_in_names = list(in_names) + list(out_names)
        if partition_name is not None:
            all_in_names.append(partition_name)

        def _body(*args):
            operands = list(args)
            if partition_name is not None:
                operands.append(bass2jax.partition_id_tensor())
            outs = bass2jax._bass_exec_p.bind(
                *operands,
                out_avals=tuple(out_avals),
                in_names=tuple(all_in_names),
                out_names=tuple(out_names),
                lowering_input_output_aliases=(),
                sim_require_finite=True,
                sim_require_nnan=True,
                nc=nc,
            )
            return tuple(outs)

        devices = jax.devices()[:n_cores]
        mesh = Mesh(np.asarray(devices), ("core",))
        in_specs = (PartitionSpec("core"),) * (n_params + n_outs)
        out_specs = (PartitionSpec("core"),) * n_outs
        donate = tuple(range(n_params, n_params + n_outs))
        fn = jax.jit(
            shard_map(
                _body, mesh=mesh, in_specs=in_specs, out_specs=out_specs,
                check_rep=False,
            ),
            donate_argnums=donate,
            keep_unused=True,
        )
        entry = (fn, in_names, out_names, out_avals)
        _nc_cache[("jit", cache_key)] = entry

    fn, in_names, out_names, out_avals = entry
    concat_in = [
        np.concatenate([np.asarray(m[name]) for m in in_maps], axis=0)
        for name in in_names
    ]
    concat_zeros = [
        np.zeros((n_cores * a.shape[0], *a.shape[1:]), a.dtype) for a in out_avals
    ]
    out_arrs = fn(*concat_in, *concat_zeros)
    return [
        {
            name: np.asarray(out_arrs[i]).reshape(n_cores, *out_avals[i].shape)[c]
            for i, name in enumerate(out_names)
        }
        for c in range(n_cores)
    ]


def build_phase1():
    """Device MLP: logits[1,S] = gelu((X@W1)^T + b1) dot W2.

    The big GEMM runs in float32r (full-rate on PE); gelu+bias on ScalarE
    straight out of PSUM; the W2 contraction is a skinny fp32 matmul.  Rows
    with |logit| near the 0 threshold are exactly recomputed on the host, so
    f32r/LUT error cannot flip a boundary decision.
    """
    if "p1" in _nc_cache:
        return _nc_cache["p1"]
    F32R = mybir.dt.float32r
    nc = bacc.Bacc()
    xt = nc.declare_dram_parameter("xt", [D, S], F32R, isOutput=False)
    w1 = nc.declare_dram_parameter("w1", [D, D], F32R, isOutput=False)
    b1p = nc.declare_dram_parameter("b1", [128, DC], F32, isOutput=False)
    w2 = nc.declare_dram_parameter("w2", [128, DC], F32R, isOutput=False)
    lg = nc.declare_dram_parameter("lg", [1, S], F32, isOutput=True)

    with tile.TileContext(nc) as tc:
        with (
            tc.tile_pool(name="sb", bufs=1) as sb,
            tc.tile_pool(name="ps", bufs=4, space="PSUM") as ps,
            tc.tile_pool(name="psl", bufs=2, space="PSUM") as psl,
        ):
            w2t = sb.tile([128, DC], F32R, tag="w2t")
            nc.sync.dma_start(out=w2t, in_=w2[:, :])
            b1t = sb.tile([128, DC], F32, tag="b1t")
            nc.sync.dma_start(out=b1t, in_=b1p[:, :])
            # W1 split by output-column chunk: the first PSUM group only needs
            # mc=0 (384 KB), not the whole 2.25 MB
            w1t = sb.tile([128, DC, D], F32R, tag="w1t")
            w1_r = w1[:, :].rearrange("(kc p) (mc m) -> p kc mc m", p=128, m=128)
            for mc in range(DC):
                nc.sync.dma_start(
                    out=w1t[:, :, mc * 128 : (mc + 1) * 128], in_=w1_r[:, :, mc, :]
                )
            NS = S // 512
            xtt = sb.tile([128, DC, NS, 512], F32R, tag="xtt")
            xt_r = xt[:, :].rearrange("(kc p) (n s) -> p kc n s", p=128, s=512)
            for n in range(NS):
                # per-s-block DMA so the first matmul group only waits for
                # W1[mc0] + one slice instead of the full transfer; the first
                # slice is further split by kc-halves to start PE earlier
                if n == 0:
                    nc.scalar.dma_start(
                        out=xtt[:, 0:3, n, :], in_=xt_r[:, 0:3, n, :]
                    )
                    nc.scalar.dma_start(
                        out=xtt[:, 3:6, n, :], in_=xt_r[:, 3:6, n, :]
                    )
                else:
                    nc.scalar.dma_start(out=xtt[:, :, n, :], in_=xt_r[:, :, n, :])
            ht = sb.tile([128, DC, S], F32R, tag="ht")
            lt = sb.tile([1, S], F32, tag="lt")
            def emit_mlp2(n):
                pl = psl.tile([1, 512], F32, tag="pl")
                for mc in range(DC):
                    nc.tensor.matmul(
                        pl,
                        w2t[:, mc : mc + 1],
                        ht[:, mc, n * 512 : (n + 1) * 512],
                        start=(mc == 0),
                        stop=(mc == DC - 1),
                    )
                nc.vector.tensor_copy(lt[:, n * 512 : (n + 1) * 512], pl)

            for n in range(NS):
                for mc in range(DC):
                    pt = ps.tile([128, 512], F32, tag="pt")
                    for kc in range(DC):
                        nc.tensor.matmul(
                            pt,
                            w1t[:, kc, mc * 128 : (mc + 1) * 128],
                            xtt[:, kc, n, :],
                            start=(kc == 0),
                            stop=(kc == DC - 1),
                        )
                    nc.scalar.activation(
                        out=ht[:, mc, n * 512 : (n + 1) * 512],
                        in_=pt,
                        func=mybir.ActivationFunctionType.Gelu,
                        bias=b1t[:, mc : mc + 1],
                        scale=1.0,
                    )
                if n >= 1:
                    emit_mlp2(n - 1)
            emit_mlp2(NS - 1)
            nc.sync.dma_start(out=lg[:, :], in_=lt)
    nc.finalize()
    _nc_cache["p1"] = nc
    return nc


def build_phase2(k_pad, kc_sis):
    """Pooled_sum [K_pad, D] = onehot[S,K_pad]^T @ X[S,D], banded over kc_sis.

    kc_sis: tuple of (kc, (si, si, ...)) — which s-chunks feed each k-chunk.
    """
    key = ("p2", k_pad, kc_sis)
    if key in _nc_cache:
        return _nc_cache[key]
    BF16 = mybir.dt.bfloat16
    nc = bacc.Bacc()
    xhp = nc.declare_dram_parameter("xh", [S, D], BF16, isOutput=False)
    xlp = nc.declare_dram_parameter("xl", [S, D], BF16, isOutput=False)
    msid = nc.declare_dram_parameter("msid", [128, SC], F32, isOutput=False)
    psum_out = nc.declare_dram_parameter("ps", [k_pad, D], F32, isOutput=True)

    # per-s-chunk k-band: contiguous kc cover of every pair this si is in
    si_band = {}
    for kc, sis in kc_sis:
        for si in sis:
            lo, hi = si_band.get(si, (kc, kc))
            si_band[si] = (min(lo, kc), max(hi, kc))
    bw = max(hi - lo + 1 for lo, hi in si_band.values())  # chunks

    with tile.TileContext(nc) as tc:
        with (
            tc.tile_pool(name="sb", bufs=1) as sb,
            tc.tile_pool(name="stg", bufs=4) as stg,
            tc.tile_pool(name="ps", bufs=4, space="PSUM") as ps,
        ):
            ms = sb.tile([128, SC], F32, tag="ms")
            nc.sync.dma_start(out=ms, in_=msid[:, :])
            xh = sb.tile([128, SC, D], BF16, tag="xh")
            xl = sb.tile([128, SC, D], BF16, tag="xl")
            xh_r = xhp[:, :].rearrange("(g si p) d -> p g si d", p=128, si=4)
            xl_r = xlp[:, :].rearrange("(g si p) d -> p g si d", p=128, si=4)
            for g in range(SC // 4):
                nc.scalar.dma_start(out=xh[:, 4 * g : 4 * g + 4, :], in_=xh_r[:, g])
                nc.scalar.dma_start(out=xl[:, 4 * g : 4 * g + 4, :], in_=xl_r[:, g])
            it = sb.tile([128, k_pad], F32, tag="it")
            nc.gpsimd.iota(
                it,
                pattern=[[1, k_pad]],
                base=0,
                channel_multiplier=0,
                allow_small_or_imprecise_dtypes=True,
            )
            oh = sb.tile([128, SC, bw * 128], BF16, tag="oh")
            for si, (lo, hi) in sorted(si_band.items()):
                w = (hi - lo + 1) * 128
                nc.vector.tensor_scalar(
                    out=oh[:, si, 0:w],
                    in0=it[:, lo * 128 : lo * 128 + w],
                    scalar1=ms[:, si : si + 1],
                    scalar2=None,
                    op0=mybir.AluOpType.is_equal,
                )
            for kc, sis in kc_sis:
                pa = ps.tile([128, 512], F32, tag="pa")
                pb = ps.tile([128, 256], F32, tag="pb")
                n_si = len(sis)
                for j, si in enumerate(sis):
                    st, sp = (j == 0), (j == n_si - 1)
                    off = (kc - si_band[si][0]) * 128
                    lhs = oh[:, si, off : off + 128]
                    nc.tensor.matmul(pa, lhs, xh[:, si, 0:512], start=st, stop=False)
                    nc.tensor.matmul(pa, lhs, xl[:, si, 0:512], start=False, stop=sp)
                    nc.tensor.matmul(pb, lhs, xh[:, si, 512:768], start=st, stop=False)
                    nc.tensor.matmul(pb, lhs, xl[:, si, 512:768], start=False, stop=sp)
                oa = stg.tile([128, D], F32, tag="oa")
                nc.vector.tensor_copy(oa[:, 0:512], pa)
                nc.vector.tensor_copy(oa[:, 512:768], pb)
                nc.sync.dma_start(
                    out=psum_out[kc * 128 : (kc + 1) * 128, :], in_=oa
                )
    nc.finalize()
    _nc_cache[key] = nc
    return nc


# |logit| band inside which the host exactly recomputes the MLP row.  Device
# logit error (f32r GEMM + gelu LUT) is measured at ~1e-4 on this workload;
# 1e-2 gives a ~100x safety margin at ~2% of rows recomputed.
TAU = np.float32(1e-2)


def _exact_rows(hidden, W1, b1, W2, b2, bb, ss):
    """Exact fp32 reference MLP for selected (batch, seq) rows."""
    import math

    from scipy.special import erf

    Xrows = hidden[bb, ss, :].astype(np.float32)  # [R, D]
    Grows = (Xrows @ W1 + b1).astype(np.float64)
    hrows = (Grows * 0.5 * (1.0 + erf(Grows / math.sqrt(2.0)))).astype(np.float32)
    return (hrows @ W2 + b2).astype(np.float32)


def _rne11(x):
    """Round fp32 to float32r's grid: round-to-nearest-even, 11 mantissa bits.

    Bit-identical to the device's fp32->f32r cast (verified on HW), so
    pre-rounding on the host and DMAing without cast preserves numerics.
    """
    b = np.ascontiguousarray(x, np.float32).view(np.uint32).astype(np.uint64)
    add = np.uint64((1 << 11) - 1)
    lsb = (b >> np.uint64(12)) & np.uint64(1)
    out = ((b + add + lsb) >> np.uint64(12) << np.uint64(12)).astype(np.uint32)
    return out.view(np.float32)


def kernel(hidden, lengths, W1, b1, W2, b2):
    hidden = np.ascontiguousarray(hidden, dtype=np.float32)
    lengths = np.asarray(lengths, dtype=np.float32)
    W1 = np.ascontiguousarray(W1, dtype=np.float32)
    b1 = np.asarray(b1, dtype=np.float32)
    W2 = np.asarray(W2, dtype=np.float32)
    b2 = np.asarray(b2, dtype=np.float32)

    # ---------------- Phase 1: logits on device ----------------
    nc1 = build_phase1()
    w1r = _rne11(W1)
    w2c = np.ascontiguousarray(_rne11(W2).reshape(DC, 128).T)
    b1c = np.ascontiguousarray(b1.reshape(DC, 128).T)
    xtr = _rne11(np.ascontiguousarray(hidden.transpose(0, 2, 1)))
    in_maps1 = [
        {"xt": xtr[b], "w1": w1r, "b1": b1c, "w2": w2c} for b in range(B)
    ]
    res1 = _run_spmd_cached("p1", nc1, in_maps1)
    logits = np.stack([res1[b]["lg"].reshape(S) for b in range(B)])  # [B,S]
    logits = (logits + b2).astype(np.float32)

    # exact host recompute of rows near the decision threshold
    band = np.abs(logits) < TAU
    if band.any():
        bb, ss = np.nonzero(band)
        logits[bb, ss] = _exact_rows(hidden, W1, b1, W2, b2, bb, ss)

    with np.errstate(over="ignore"):
        probs = (np.float32(1.0) / (np.float32(1.0) + np.exp(-logits))).astype(
            np.float32
        )

    actual_lens = (lengths * np.float32(S)).astype(np.int32)  # [B]
    sidx = np.arange(S, dtype=np.int64)
    valid = (sidx[None, :] < actual_lens[:, None]).astype(np.float32)  # [B,S]
    soft = probs * valid
    hard = (probs > np.float32(0.5)).astype(np.float32) * valid
    last_valid = np.clip(actual_lens - 1, 0, S - 1)
    bi = np.arange(B)
    soft[bi, last_valid] = np.float32(1.0)
    hard[bi, last_valid] = np.float32(1.0)
    hard_b = (hard - soft) + soft  # exact == hard in fp32; mirrors reference
    K = int(np.max(np.sum(hard_b, axis=1)))
    seg_id = np.cumsum(hard_b, axis=1, dtype=np.float32) - hard_b  # [B,S]
    nb = np.sum(hard_b * valid, axis=1)  # [B] f32

    masked_probs = probs * valid

    max_segments = max(K, 1)
    full = nb >= max_segments - 1
    partial = (nb > 0) & (nb < max_segments - 1)
    shortened = np.where(
        full,
        np.float32(1.0),
        np.where(partial, (nb + np.float32(1.0)) / np.float32(max_segments),
                 np.float32(0.0)),
    ).astype(np.float32)

    # ---------------- Phase 2: banded one-hot segment-sum ----------------
    k_pad = ((K + 127) // 128) * 128
    msid = np.where(valid > 0, seg_id, np.float32(-1.0)).astype(np.float32)

    kc_map = {}
    for b in range(B):
        al = int(actual_lens[b])
        for si in range(SC):
            s0 = si * 128
            if s0 >= al:
                break
            s1 = min(s0 + 128, al)
            lo = int(seg_id[b, s0])
            hi = int(seg_id[b, s1 - 1])
            for kc in range(lo // 128, hi // 128 + 1):
                kc_map.setdefault(kc, set()).add(si)
    kc_sis = tuple(
        (kc, tuple(sorted(kc_map[kc]))) for kc in sorted(kc_map)
    )

    nc2 = build_phase2(k_pad, kc_sis)
    import ml_dtypes

    xh_all = hidden.astype(ml_dtypes.bfloat16)
    xl_all = (hidden - xh_all.astype(np.float32)).astype(ml_dtypes.bfloat16)
    in_maps2 = [
        {"xh": xh_all[b], "xl": xl_all[b],
         "msid": np.ascontiguousarray(msid[b].reshape(SC, 128).T)}
        for b in range(B)
    ]
    global _last_in_maps1, _last_in_maps2
    _last_in_maps1 = in_maps1
    _last_in_maps2 = in_maps2
    res2 = _run_spmd_cached(("p2", k_pad, kc_sis), nc2, in_maps2)
    pooled_sum = np.stack([res2[b]["ps"][:K] for b in range(B)])  # [B,K,D]

    counts = np.zeros((B, K), dtype=np.float32)
    for b in range(B):
        v = valid[b] > 0
        ids = seg_id[b, v].astype(np.int64)
        if ids.size:
            cnt = np.bincount(ids, minlength=K)
            counts[b] = cnt[:K]
    counts = np.maximum(counts, np.float32(1.0))
    pooled = pooled_sum / counts[:, :, None]

    return (
        pooled.astype(np.float32),
        masked_probs.astype(np.float32),
        shortened,
        nb.astype(np.float32),
        actual_lens.astype(np.float32),
    )


# revision 23
# speedup vs baseline: 1.4399x; 1.0677x over previous
"""Trainium2 Bass kernel for nn_BoundaryPredictorMLP (segment-mean pooling MLP).

Sharding: pure data parallel — one batch sample per NeuronCore (B=8, 8 cores).

Pipeline (two device launches with a host step between — the number of
segments K is data-dependent, so the pooling kernel's shapes/band structure
are only known after the MLP):

  Launch 1 (device): logits = gelu((X @ W1)^T + b1) . W2 per core.  The big
    GEMM runs in float32r (full-rate 1 cyc/row on PE vs fp32's 4; f32r = fp32
    rounded to 11 mantissa bits, pre-rounded on host so plain HWDGE DMAs
    work); gelu+bias on ScalarE straight out of PSUM; W2 contraction is a
    skinny f32r matmul software-pipelined one s-block behind the GEMM.
  Host: sigmoid / boundary extraction in numpy mirroring reference fp32
    arithmetic.  Rows with |logit| < TAU are recomputed exactly (fp32 BLAS +
    fp64 erf-gelu), so f32r/gelu-LUT error (~2e-4, TAU/err ~ 50x margin)
    cannot flip a probs>0.5 boundary decision — segment structure is exact.
  Launch 2 (device): segment-sum pooling as a one-hot matmul.  X is split
    hi/lo into two bf16 halves on host (exact products with the 0/1 one-hot,
    fp32 PSUM accumulate -> ~1e-6 pooled error at bf16 speed).  The one-hot
    [S, K_pad] is built on device from masked seg-ids via iota +
    per-partition is_equal, only over each s-chunk's reachable k-band, and
    only (s-chunk, k-chunk) matmuls that can be non-zero on some core are
    emitted (segments are contiguous -> banded).
  Host: divide by counts, assemble the 5 reference outputs.
"""

import numpy as np

import concourse.bacc as bacc
import concourse.tile as tile
from concourse import mybir

F32 = mybir.dt.float32

B, S, D = 8, 2048, 768
NCORES = 8
SC = S // 128  # 16 s-chunks
DC = D // 128  # 6 d-chunks

_nc_cache = {}


def _run_spmd_cached(cache_key, nc, in_maps):
    """run_bass_via_pjrt with the jitted executable cached across calls.

    Mirrors concourse.bass2jax.run_bass_via_pjrt's multi-core path (shard_map
    over 8 devices, inputs concatenated on axis 0, donated zero output
    buffers) but keeps the compiled callable so repeat kernel() invocations
    skip retracing/recompiling.
    """
    import jax
    from jax.sharding import Mesh, PartitionSpec
    from jax.experimental.shard_map import shard_map
    from concourse import bass2jax, mybir as _mb

    n_cores = len(in_maps)
    entry = _nc_cache.get(("jit", cache_key))
    if entry is None:
        bass2jax.install_neuronx_cc_hook()
        in_names, out_names, out_avals = [], [], []
        partition_name = (
            nc.partition_id_tensor.name if nc.partition_id_tensor else None
        )
        for alloc in nc.m.functions[0].allocations:
            if not isinstance(alloc, _mb.MemoryLocationSet):
                continue
            name = alloc.memorylocations[0].name
            if alloc.kind == "ExternalInput":
                if name != partition_name:
                    in_names.append(name)
            elif alloc.kind == "ExternalOutput":
                out_names.append(name)
                out_avals.append(
                    jax.core.ShapedArray(
                        tuple(alloc.tensor_shape), _mb.dt.np(alloc.dtype)
                    )
                )
        n_params = len(in_names)
        n_outs = len(out_avals)
        all_in_names = list(in_names) + list(out_names)
        if partition_name is not None:
            all_in_names.append(partition_name)

        def _body(*args):
            operands = list(args)
            if partition_name is not None:
                operands.append(bass2jax.partition_id_tensor())
            outs = bass2jax._bass_exec_p.bind(
                *operands,
                out_avals=tuple(out_avals),
                in_names=tuple(all_in_names),
                out_names=tuple(out_names),
                lowering_input_output_aliases=(),
                sim_require_finite=True,
                sim_require_nnan=True,
                nc=nc,
            )
            return tuple(outs)

        devices = jax.devices()[:n_cores]
        mesh = Mesh(np.asarray(devices), ("core",))
        in_specs = (PartitionSpec("core"),) * (n_params + n_outs)
        out_specs = (PartitionSpec("core"),) * n_outs
        donate = tuple(range(n_params, n_params + n_outs))
        fn = jax.jit(
            shard_map(
                _body, mesh=mesh, in_specs=in_specs, out_specs=out_specs,
                check_rep=False,
            ),
            donate_argnums=donate,
            keep_unused=True,
        )
        entry = (fn, in_names, out_names, out_avals)
        _nc_cache[("jit", cache_key)] = entry

    fn, in_names, out_names, out_avals = entry
    concat_in = [
        np.concatenate([np.asarray(m[name]) for m in in_maps], axis=0)
        for name in in_names
    ]
    concat_zeros = [
        np.zeros((n_cores * a.shape[0], *a.shape[1:]), a.dtype) for a in out_avals
    ]
    out_arrs = fn(*concat_in, *concat_zeros)
    return [
        {
            name: np.asarray(out_arrs[i]).reshape(n_cores, *out_avals[i].shape)[c]
            for i, name in enumerate(out_names)
        }
        for c in range(n_cores)
    ]


def build_phase1():
    """Device MLP: logits[1,S] = gelu((X@W1)^T + b1) dot W2.

    The big GEMM runs in float32r (full-rate on PE); gelu+bias on ScalarE
    straight out of PSUM; the W2 contraction is a skinny fp32 matmul.  Rows
    with |logit| near the 0 threshold are exactly recomputed on the host, so
    f32r/LUT error cannot flip a boundary decision.
    """
    if "p1" in _nc_cache:
        return _nc_cache["p1"]
    F32R = mybir.dt.float32r
    nc = bacc.Bacc()
    xt = nc.declare_dram_parameter("xt", [D, S], F32R, isOutput=False)
    w1 = nc.declare_dram_parameter("w1", [D, D], F32R, isOutput=False)
    b1p = nc.declare_dram_parameter("b1", [128, DC], F32, isOutput=False)
    w2 = nc.declare_dram_parameter("w2", [128, DC], F32R, isOutput=False)
    lg = nc.declare_dram_parameter("lg", [1, S], F32, isOutput=True)

    with tile.TileContext(nc) as tc:
        with (
            tc.tile_pool(name="sb", bufs=1) as sb,
            tc.tile_pool(name="ps", bufs=4, space="PSUM") as ps,
            tc.tile_pool(name="psl", bufs=2, space="PSUM") as psl,
        ):
            w2t = sb.tile([128, DC], F32R, tag="w2t")
            nc.sync.dma_start(out=w2t, in_=w2[:, :])
            b1t = sb.tile([128, DC], F32, tag="b1t")
            nc.sync.dma_start(out=b1t, in_=b1p[:, :])
            # PE warm-up: dummy matmuls during the DMA lead-in keep the PE
            # HAM activity window busy so real matmuls start at 2.4 GHz
            dum_l = sb.tile([128, 128], F32, tag="dum_l")
            dum_r = sb.tile([128, 256], F32, tag="dum_r")
            nc.vector.memset(dum_l, 0.0)
            nc.vector.memset(dum_r, 0.0)
            for _ in range(7):
                pd = psl.tile([128, 256], F32, tag="pd")
                nc.tensor.matmul(pd, dum_l, dum_r, start=True, stop=True)

            # All large transfers serialized on ONE ring in the order PE
            # consumes them (x^T slice 0, W1 col-chunks 1..5, x^T 1..3);
            # only the small W1[mc0] rides the other ring so the first PSUM
            # group is ready ~4.7us in.  The warm-up matmuls above bridge
            # exactly that window so real matmuls start at 2.4 GHz.
            w1t = sb.tile([128, DC, D], F32R, tag="w1t")
            w1_r = w1[:, :].rearrange("(kc p) (mc m) -> p kc mc m", p=128, m=128)
            NS = S // 512
            xtt = sb.tile([128, DC, NS, 512], F32R, tag="xtt")
            xt_r = xt[:, :].rearrange("(kc p) (n s) -> p kc n s", p=128, s=512)
            nc.sync.dma_start(out=w1t[:, :, 0:128], in_=w1_r[:, :, 0, :])
            nc.scalar.dma_start(out=xtt[:, :, 0, :], in_=xt_r[:, :, 0, :])
            for mc in range(1, DC):
                nc.scalar.dma_start(
                    out=w1t[:, :, mc * 128 : (mc + 1) * 128], in_=w1_r[:, :, mc, :]
                )
            for n in range(1, NS):
                nc.scalar.dma_start(out=xtt[:, :, n, :], in_=xt_r[:, :, n, :])
            ht = sb.tile([128, DC, S], F32R, tag="ht")
            lt = sb.tile([1, S], F32, tag="lt")
            def emit_mlp2(n):
                pl = psl.tile([1, 512], F32, tag="pl")
                for mc in range(DC):
                    nc.tensor.matmul(
                        pl,
                        w2t[:, mc : mc + 1],
                        ht[:, mc, n * 512 : (n + 1) * 512],
                        start=(mc == 0),
                        stop=(mc == DC - 1),
                    )
                nc.vector.tensor_copy(lt[:, n * 512 : (n + 1) * 512], pl)

            for n in range(NS):
                for mc in range(DC):
                    pt = ps.tile([128, 512], F32, tag="pt")
                    for kc in range(DC):
                        nc.tensor.matmul(
                            pt,
                            w1t[:, kc, mc * 128 : (mc + 1) * 128],
                            xtt[:, kc, n, :],
                            start=(kc == 0),
                            stop=(kc == DC - 1),
                        )
                    nc.scalar.activation(
                        out=ht[:, mc, n * 512 : (n + 1) * 512],
                        in_=pt,
                        func=mybir.ActivationFunctionType.Gelu,
                        bias=b1t[:, mc : mc + 1],
                        scale=1.0,
                    )
                if n >= 1:
                    emit_mlp2(n - 1)
            emit_mlp2(NS - 1)
            nc.sync.dma_start(out=lg[:, :], in_=lt)
    nc.finalize()
    _nc_cache["p1"] = nc
    return nc


def build_phase2(k_pad, kc_sis):
    """Pooled_sum [K_pad, D] = onehot[S,K_pad]^T @ X[S,D], banded over kc_sis.

    kc_sis: tuple of (kc, (si, si, ...)) — which s-chunks feed each k-chunk.
    """
    key = ("p2", k_pad, kc_sis)
    if key in _nc_cache:
        return _nc_cache[key]
    BF16 = mybir.dt.bfloat16
    nc = bacc.Bacc()
    xhp = nc.declare_dram_parameter("xh", [S, D], BF16, isOutput=False)
    xlp = nc.declare_dram_parameter("xl", [S, D], BF16, isOutput=False)
    msid = nc.declare_dram_parameter("msid", [128, SC], F32, isOutput=False)
    psum_out = nc.declare_dram_parameter("ps", [k_pad, D], F32, isOutput=True)

    # per-s-chunk k-band: contiguous kc cover of every pair this si is in
    si_band = {}
    for kc, sis in kc_sis:
        for si in sis:
            lo, hi = si_band.get(si, (kc, kc))
            si_band[si] = (min(lo, kc), max(hi, kc))
    bw = max(hi - lo + 1 for lo, hi in si_band.values())  # chunks

    with tile.TileContext(nc) as tc:
        with (
            tc.tile_pool(name="sb", bufs=1) as sb,
            tc.tile_pool(name="stg", bufs=4) as stg,
            tc.tile_pool(name="ps", bufs=3, space="PSUM") as ps,
            tc.tile_pool(name="psd", bufs=2, space="PSUM") as psd,
        ):
            ms = sb.tile([128, SC], F32, tag="ms")
            nc.sync.dma_start(out=ms, in_=msid[:, :])
            dum_l = sb.tile([128, 128], F32, tag="dum_l")
            dum_r = sb.tile([128, 256], F32, tag="dum_r")
            nc.vector.memset(dum_l, 0.0)
            nc.vector.memset(dum_r, 0.0)
            for _ in range(4):
                pd = psd.tile([128, 256], F32, tag="pd")
                nc.tensor.matmul(pd, dum_l, dum_r, start=True, stop=True)
            xh = sb.tile([128, SC, D], BF16, tag="xh")
            xl = sb.tile([128, SC, D], BF16, tag="xl")
            xh_r = xhp[:, :].rearrange("(g si p) d -> p g si d", p=128, si=4)
            xl_r = xlp[:, :].rearrange("(g si p) d -> p g si d", p=128, si=4)
            for g in range(SC // 4):
                nc.scalar.dma_start(out=xh[:, 4 * g : 4 * g + 4, :], in_=xh_r[:, g])
                nc.scalar.dma_start(out=xl[:, 4 * g : 4 * g + 4, :], in_=xl_r[:, g])
            it = sb.tile([128, k_pad], F32, tag="it")
            nc.gpsimd.iota(
                it,
                pattern=[[1, k_pad]],
                base=0,
                channel_multiplier=0,
                allow_small_or_imprecise_dtypes=True,
            )
            oh = sb.tile([128, SC, bw * 128], BF16, tag="oh")
            for si, (lo, hi) in sorted(si_band.items()):
                w = (hi - lo + 1) * 128
                nc.vector.tensor_scalar(
                    out=oh[:, si, 0:w],
                    in0=it[:, lo * 128 : lo * 128 + w],
                    scalar1=ms[:, si : si + 1],
                    scalar2=None,
                    op0=mybir.AluOpType.is_equal,
                )
            for kc, sis in kc_sis:
                pa = ps.tile([128, 512], F32, tag="pa")
                pb = ps.tile([128, 256], F32, tag="pb")
                n_si = len(sis)
                for j, si in enumerate(sis):
                    st, sp = (j == 0), (j == n_si - 1)
                    off = (kc - si_band[si][0]) * 128
                    lhs = oh[:, si, off : off + 128]
                    nc.tensor.matmul(pa, lhs, xh[:, si, 0:512], start=st, stop=False)
                    nc.tensor.matmul(pa, lhs, xl[:, si, 0:512], start=False, stop=sp)
                    nc.tensor.matmul(pb, lhs, xh[:, si, 512:768], start=st, stop=False)
                    nc.tensor.matmul(pb, lhs, xl[:, si, 512:768], start=False, stop=sp)
                oa = stg.tile([128, D], F32, tag="oa")
                nc.vector.tensor_copy(oa[:, 0:512], pa)
                nc.vector.tensor_copy(oa[:, 512:768], pb)
                nc.sync.dma_start(
                    out=psum_out[kc * 128 : (kc + 1) * 128, :], in_=oa
                )
    nc.finalize()
    _nc_cache[key] = nc
    return nc


# |logit| band inside which the host exactly recomputes the MLP row.  Device
# logit error (f32r GEMM + gelu LUT) is measured at ~1e-4 on this workload;
# 1e-2 gives a ~100x safety margin at ~2% of rows recomputed.
TAU = np.float32(1e-2)


def _exact_rows(hidden, W1, b1, W2, b2, bb, ss):
    """Exact fp32 reference MLP for selected (batch, seq) rows."""
    import math

    from scipy.special import erf

    Xrows = hidden[bb, ss, :].astype(np.float32)  # [R, D]
    Grows = (Xrows @ W1 + b1).astype(np.float64)
    hrows = (Grows * 0.5 * (1.0 + erf(Grows / math.sqrt(2.0)))).astype(np.float32)
    return (hrows @ W2 + b2).astype(np.float32)


def _rne11(x):
    """Round fp32 to float32r's grid: round-to-nearest-even, 11 mantissa bits.

    Bit-identical to the device's fp32->f32r cast (verified on HW), so
    pre-rounding on the host and DMAing without cast preserves numerics.
    """
    b = np.ascontiguousarray(x, np.float32).view(np.uint32).astype(np.uint64)
    add = np.uint64((1 << 11) - 1)
    lsb = (b >> np.uint64(12)) & np.uint64(1)
    out = ((b + add + lsb) >> np.uint64(12) << np.uint64(12)).astype(np.uint32)
    return out.view(np.float32)


def kernel(hidden, lengths, W1, b1, W2, b2):
    hidden = np.ascontiguousarray(hidden, dtype=np.float32)
    lengths = np.asarray(lengths, dtype=np.float32)
    W1 = np.ascontiguousarray(W1, dtype=np.float32)
    b1 = np.asarray(b1, dtype=np.float32)
    W2 = np.asarray(W2, dtype=np.float32)
    b2 = np.asarray(b2, dtype=np.float32)

    # ---------------- Phase 1: logits on device ----------------
    nc1 = build_phase1()
    w1r = _rne11(W1)
    w2c = np.ascontiguousarray(_rne11(W2).reshape(DC, 128).T)
    b1c = np.ascontiguousarray(b1.reshape(DC, 128).T)
    xtr = _rne11(np.ascontiguousarray(hidden.transpose(0, 2, 1)))
    in_maps1 = [
        {"xt": xtr[b], "w1": w1r, "b1": b1c, "w2": w2c} for b in range(B)
    ]
    res1 = _run_spmd_cached("p1", nc1, in_maps1)
    logits = np.stack([res1[b]["lg"].reshape(S) for b in range(B)])  # [B,S]
    logits = (logits + b2).astype(np.float32)

    # exact host recompute of rows near the decision threshold
    band = np.abs(logits) < TAU
    if band.any():
        bb, ss = np.nonzero(band)
        logits[bb, ss] = _exact_rows(hidden, W1, b1, W2, b2, bb, ss)

    with np.errstate(over="ignore"):
        probs = (np.float32(1.0) / (np.float32(1.0) + np.exp(-logits))).astype(
            np.float32
        )

    actual_lens = (lengths * np.float32(S)).astype(np.int32)  # [B]
    sidx = np.arange(S, dtype=np.int64)
    valid = (sidx[None, :] < actual_lens[:, None]).astype(np.float32)  # [B,S]
    soft = probs * valid
    hard = (probs > np.float32(0.5)).astype(np.float32) * valid
    last_valid = np.clip(actual_lens - 1, 0, S - 1)
    bi = np.arange(B)
    soft[bi, last_valid] = np.float32(1.0)
    hard[bi, last_valid] = np.float32(1.0)
    hard_b = (hard - soft) + soft  # exact == hard in fp32; mirrors reference
    K = int(np.max(np.sum(hard_b, axis=1)))
    seg_id = np.cumsum(hard_b, axis=1, dtype=np.float32) - hard_b  # [B,S]
    nb = np.sum(hard_b * valid, axis=1)  # [B] f32

    masked_probs = probs * valid

    max_segments = max(K, 1)
    full = nb >= max_segments - 1
    partial = (nb > 0) & (nb < max_segments - 1)
    shortened = np.where(
        full,
        np.float32(1.0),
        np.where(partial, (nb + np.float32(1.0)) / np.float32(max_segments),
                 np.float32(0.0)),
    ).astype(np.float32)

    # ---------------- Phase 2: banded one-hot segment-sum ----------------
    k_pad = ((K + 127) // 128) * 128
    msid = np.where(valid > 0, seg_id, np.float32(-1.0)).astype(np.float32)

    kc_map = {}
    for b in range(B):
        al = int(actual_lens[b])
        for si in range(SC):
            s0 = si * 128
            if s0 >= al:
                break
            s1 = min(s0 + 128, al)
            lo = int(seg_id[b, s0])
            hi = int(seg_id[b, s1 - 1])
            for kc in range(lo // 128, hi // 128 + 1):
                kc_map.setdefault(kc, set()).add(si)
    kc_sis = tuple(
        (kc, tuple(sorted(kc_map[kc]))) for kc in sorted(kc_map)
    )

    nc2 = build_phase2(k_pad, kc_sis)
    import ml_dtypes

    xh_all = hidden.astype(ml_dtypes.bfloat16)
    xl_all = (hidden - xh_all.astype(np.float32)).astype(ml_dtypes.bfloat16)
    in_maps2 = [
        {"xh": xh_all[b], "xl": xl_all[b],
         "msid": np.ascontiguousarray(msid[b].reshape(SC, 128).T)}
        for b in range(B)
    ]
    global _last_in_maps1, _last_in_maps2
    _last_in_maps1 = in_maps1
    _last_in_maps2 = in_maps2
    res2 = _run_spmd_cached(("p2", k_pad, kc_sis), nc2, in_maps2)
    pooled_sum = np.stack([res2[b]["ps"][:K] for b in range(B)])  # [B,K,D]

    counts = np.zeros((B, K), dtype=np.float32)
    for b in range(B):
        v = valid[b] > 0
        ids = seg_id[b, v].astype(np.int64)
        if ids.size:
            cnt = np.bincount(ids, minlength=K)
            counts[b] = cnt[:K]
    counts = np.maximum(counts, np.float32(1.0))
    pooled = pooled_sum / counts[:, :, None]

    return (
        pooled.astype(np.float32),
        masked_probs.astype(np.float32),
        shortened,
        nb.astype(np.float32),
        actual_lens.astype(np.float32),
    )
